# revision 22
# baseline (speedup 1.0000x reference)
"""Trainium2 8-core GATv2 message-passing kernel (nn_AtomGraphEncoder).

Dense (block, round, lane) design:
- Nodes sharded 8 x 12500 by id; WITHIN each core, nodes are permuted into
  blocks of 128 lanes with an SPMD-uniform per-class template R[b, k]
  (class k of an edge = src // 25000 = core-pair window, int16-gatherable).
- Per layer: proj fs/fd on PE (fd -> DRAM fd_tab, fs -> AllGather fs_full),
  then per superblock: one dma_gather per class window, dense free-dim ops:
  u = Prelu(fsg + fd bcast), sign-split reduces for GATv2 logits (|a| folded
  into W on host), exp (+pad mask -1e30), m = [ex*fs | ex], per-(b,k) strided
  reduce over rounds + per-block class-sum -> normalize -> hT / out.
- No scatter-add, no DRAM accumulators, no fd gather.
- Host: weight folding (undone on output), node permutation (undone on host).
"""
import sys

import numpy as np
import ml_dtypes

sys.path.insert(0, '/opt/trn_rl_repo')

N, E = 100000, 400000
ATOM_DIM, HID, LAYERS, HEADS = 74, 256, 3, 4
OUT = HID // HEADS
NCORES = 8
NPC = N // NCORES
W = 25000
NCLS = 4
BF = ml_dtypes.bfloat16
EPS = 1e-20
MASKNEG = -1e30


def _bf(x):
    return np.asarray(x).astype(BF)


def _fold_weights(W_in, b_in, W_src, b_src, W_dst, b_dst, attn, bias):
    Ts, Tinvs = [], []
    pos_cnt = np.zeros((LAYERS, HEADS), np.int64)
    zero_cnt = np.zeros((LAYERS, HEADS), np.int64)
    for l in range(LAYERS):
        Tl = np.zeros((HID, HID), np.float64)
        Tinv = np.zeros((HID, HID), np.float64)
        for h in range(HEADS):
            a = np.asarray(attn)[l, h].astype(np.float64)
            order = np.concatenate([
                np.where(a > 0)[0], np.where(a == 0)[0], np.where(a < 0)[0]])
            pos_cnt[l, h] = (a > 0).sum()
            zero_cnt[l, h] = (a == 0).sum()
            for j, p in enumerate(order):
                s = abs(a[p]) if a[p] != 0 else 1.0
                Tl[h * OUT + p, h * OUT + j] = s
                Tinv[h * OUT + j, h * OUT + p] = 1.0 / s
        Ts.append(Tl)
        Tinvs.append(Tinv)
    Ws_eff, Wd_eff, bs_eff, bd_eff = [], [], [], []
    for l in range(LAYERS):
        Tp = np.eye(HID) if l == 0 else Tinvs[l - 1]
        Ws = np.asarray(W_src)[l].astype(np.float64)
        Wd = np.asarray(W_dst)[l].astype(np.float64)
        bprev = np.zeros(HID) if l == 0 else np.asarray(bias)[l - 1].astype(np.float64)
        Ws_eff.append((Tp @ Ws @ Ts[l]).astype(np.float32))
        Wd_eff.append((Tp @ Wd @ Ts[l]).astype(np.float32))
        bs_eff.append(((np.asarray(b_src)[l] + bprev @ Ws) @ Ts[l]).astype(np.float32))
        bd_eff.append(((np.asarray(b_dst)[l] + bprev @ Wd) @ Ts[l]).astype(np.float32))
    return Ws_eff, Wd_eff, bs_eff, bd_eff, pos_cnt, zero_cnt, Tinvs[-1]


# ---------------------------------------------------------------- host prep
CMAX = 32           # max columns per superblock tile
SBMAXB = 8          # max blocks per superblock


def _percore_edges(src, dst):
    out = []
    for c in range(NCORES):
        m = (dst >= c * NPC) & (dst < (c + 1) * NPC)
        out.append((src[m], dst[m] - c * NPC))
    return out


def _class_counts(es, ed):
    cnt = np.zeros((NPC, NCLS), np.int64)
    np.add.at(cnt, (ed, es // W), 1)
    return cnt


def _lex_order(cnt):
    return np.lexsort((-cnt[:, 0], -cnt[:, 1], -cnt[:, 2], -cnt[:, 3]))


def _block_R(cnt, order, nb):
    cs = cnt[order]
    cs = np.vstack([cs, np.zeros((nb * 128 - len(cs), NCLS), np.int64)])
    return cs.reshape(nb, 128, NCLS).max(1)


def build_structure(src, dst, quant=0.45):
    src = np.asarray(src).astype(np.int64)
    dst = np.asarray(dst).astype(np.int64)
    percore = _percore_edges(src, dst)
    cnts = [_class_counts(es, ed) for es, ed in percore]
    NB = (NPC + 127) // 128

    Rs = np.stack([_block_R(cnts[c], _lex_order(cnts[c]), NB) for c in range(NCORES)])
    Rtemp = np.ceil(np.quantile(Rs, quant, axis=0)).astype(np.int64)

    assigns = np.full((NCORES, NB, 128), -1, np.int64)
    overflows = []
    for c in range(NCORES):
        cnt = cnts[c]
        order = _lex_order(cnt)
        cap = np.full(NB, 128, np.int64)
        fill = np.zeros(NB, np.int64)
        ov = []
        for idx in order:
            v = cnt[idx]
            placed = False
            for b in range(NB):
                if cap[b] > 0 and (Rtemp[b] >= v).all():
                    assigns[c, b, fill[b]] = idx
                    fill[b] += 1
                    cap[b] -= 1
                    placed = True
                    break
            if not placed:
                ov.append(idx)
        overflows.append(ov)

    nov_nodes = max(len(o) for o in overflows)
    n_ovb = (nov_nodes + 127) // 128
    if n_ovb:
        Rov = np.zeros((n_ovb, NCLS), np.int64)
        ovassign = np.full((NCORES, n_ovb, 128), -1, np.int64)
        for c in range(NCORES):
            ov = overflows[c]
            if ov:
                ovc = cnts[c][ov]
                o2 = np.lexsort((-ovc[:, 0], -ovc[:, 1], -ovc[:, 2], -ovc[:, 3]))
                ov = [ov[i] for i in o2]
            for i, idx in enumerate(ov):
                b = i // 128
                ovassign[c, b, i % 128] = idx
                np.maximum(Rov[b], cnts[c][idx], out=Rov[b])
        Rtemp = np.vstack([Rtemp, Rov])
        assigns = np.concatenate([assigns, ovassign], axis=1)
    return Rtemp, assigns


def build_slots(src, dst, Rtemp, assigns):
    src = np.asarray(src).astype(np.int64)
    dst = np.asarray(dst).astype(np.int64)
    NBE = Rtemp.shape[0]
    SLAB = NBE * 128

    posmap = np.full((NCORES, NPC), -1, np.int64)
    for c in range(NCORES):
        b_idx, lane_idx = np.nonzero(assigns[c] >= 0)
        nodes = assigns[c][b_idx, lane_idx]
        posmap[c, nodes] = b_idx * 128 + lane_idx

    core_of = np.arange(N) // NPC
    fsrow = core_of * SLAB + posmap[core_of, np.arange(N) % NPC]

    percore = _percore_edges(src, dst)

    Rsum = Rtemp.sum(1)
    sbs = []
    cur, cur_cols = [], 0
    for b in range(NBE):
        if Rsum[b] == 0:
            continue
        if cur and (cur_cols + Rsum[b] > CMAX or len(cur) == SBMAXB):
            sbs.append(cur)
            cur, cur_cols = [], 0
        cur.append(b)
        cur_cols += Rsum[b]
    if cur:
        sbs.append(cur)
    empty_blocks = [b for b in range(NBE) if Rsum[b] == 0]

    sb_meta = []
    gcol = 0
    for blocks in sbs:
        runs = []
        c0 = gcol
        for k in range(NCLS):
            for b in blocks:
                R = int(Rtemp[b, k])
                if R:
                    runs.append((k, b, R, gcol))
                    gcol += R
        sb_meta.append(dict(blocks=blocks, runs=runs, col0=c0, ncols=gcol - c0))
    C = gcol
    S = C * 128

    fsidx = np.zeros((NCORES, S), np.int64)
    mask = np.zeros((NCORES, 128, C), np.float32)
    for c in range(NCORES):
        es, ed = percore[c]
        cls = es // W
        order = np.lexsort((cls, ed))
        es_s, cls_s = es[order], cls[order]
        ed_s = ed[order]
        key = ed_s * NCLS + cls_s
        starts = np.searchsorted(key, np.arange(NPC * NCLS))
        ends = np.searchsorted(key, np.arange(NPC * NCLS), side='right')
        for sb in sb_meta:
            for (k, b, R, g0) in sb['runs']:
                for lane in range(128):
                    node = assigns[c, b, lane]
                    if node >= 0:
                        s0, s1 = starts[node * NCLS + k], ends[node * NCLS + k]
                        nn = min(R, s1 - s0)
                        for r in range(nn):
                            fsidx[c, (g0 + r) * 128 + lane] = \
                                fsrow[es_s[s0 + r]] - k * 2 * SLAB
                        for r in range(nn, R):
                            mask[c, lane, g0 + r] = MASKNEG
                    else:
                        for r in range(R):
                            mask[c, lane, g0 + r] = MASKNEG
    return dict(NBE=NBE, sbs=sb_meta, empty_blocks=empty_blocks, C=C, S=S,
                fsidx=fsidx, mask=mask, posmap=posmap)


def _wrap16(idx):
    w = np.ascontiguousarray(np.asarray(idx).reshape(-1, 16).T).astype(np.int16)
    return np.tile(w, (8, 1))


# ---------------------------------------------------------------- bass build
def _build(slots, pos_cnt, zero_cnt):
    import concourse.bass as bass
    import concourse.tile as tile
    from concourse import bacc, mybir, library_config

    NBE, sbs, C, S = slots['NBE'], slots['sbs'], slots['C'], slots['S']
    empty_blocks = slots['empty_blocks']
    SLAB = NBE * 128
    ALPHA = 0.2

    nc = bacc.Bacc("TRN2", target_bir_lowering=False, debug=False,
                   num_devices=NCORES)
    dt = mybir.dt
    atomT_d = nc.dram_tensor("atomT", [ATOM_DIM + 1, SLAB], dt.bfloat16,
                             kind="ExternalInput")
    win_d = nc.dram_tensor("win", [ATOM_DIM + 1, HID], dt.bfloat16,
                           kind="ExternalInput")
    wsd_d = nc.dram_tensor("wsd", [128, 2 * LAYERS, 512], dt.bfloat16,
                           kind="ExternalInput")
    fsi_d = nc.dram_tensor("fsi", [128, S // 16], dt.int16, kind="ExternalInput")
    mask_d = nc.dram_tensor("maskt", [128, C], dt.bfloat16, kind="ExternalInput")
    ident_d = nc.dram_tensor("ident", [128, 128], dt.bfloat16, kind="ExternalInput")
    out_d = nc.dram_tensor("out", [SLAB, HID], dt.float32, kind="ExternalOutput")

    fs_bounce = [nc.dram_tensor(f"fs_bounce{i}", [SLAB, HID], dt.bfloat16)
                 for i in range(2)]
    fd_tab = [nc.dram_tensor(f"fd_tab{i}", [SLAB, HID], dt.bfloat16)
              for i in range(2)]
    fs_full = [nc.dram_tensor(f"fs_full{i}", [NCORES * SLAB, HID], dt.bfloat16,
                              addr_space="Shared") for i in range(2)]

    with tile.TileContext(nc) as tc:
        nc.gpsimd.load_library(library_config.mlp)
        with tc.tile_pool(name="persist", bufs=1) as pp, \
             tc.tile_pool(name="gat", bufs=3) as gp, \
             tc.tile_pool(name="fdp", bufs=2) as fp, \
             tc.tile_pool(name="upool", bufs=2) as up, \
             tc.tile_pool(name="cmp", bufs=1) as cp, \
             tc.tile_pool(name="blk", bufs=2) as bp, \
             tc.tile_pool(name="stage", bufs=2) as sp, \
             tc.tile_pool(name="psA", bufs=2, space="PSUM") as psA, \
             tc.tile_pool(name="psT", bufs=2, space="PSUM") as psT:

            fsi = pp.tile([128, S // 16], dt.int16, tag="fsi")
            maskt = pp.tile([128, C], dt.bfloat16, tag="maskt")
            wsd = pp.tile([128, 2 * LAYERS, 512], dt.bfloat16, tag="wsd")
            win = pp.tile([ATOM_DIM + 1, HID], dt.bfloat16, tag="win")
            ident = pp.tile([128, 128], dt.bfloat16, tag="ident")
            hT = pp.tile([128, 2, SLAB], dt.bfloat16, tag="hT")
            nc.sync.dma_start(fsi[:], fsi_d[:])
            nc.sync.dma_start(maskt[:], mask_d[:])
            nc.sync.dma_start(wsd[:], wsd_d[:])
            nc.sync.dma_start(win[:], win_d[:])
            nc.sync.dma_start(ident[:], ident_d[:])

            # ---- input projection: hT0 = (atomT^T @ Win)^T (chunk-streamed)
            CH = 1024
            s0 = 0
            while s0 < SLAB:
                w_all = min(CH, SLAB - s0)
                atc = sp.tile([ATOM_DIM + 1, CH], dt.bfloat16, tag="atc")
                nc.sync.dma_start(atc[:, 0:w_all], atomT_d[:, s0:s0 + w_all])
                t = 0
                while t < w_all:
                    w_ = min(512, w_all - t)
                    for cch in range(2):
                        ps = psA.tile([128, 512], dt.float32, tag="projps")
                        nc.tensor.matmul(ps[:, 0:w_],
                                         win[:, cch * 128:(cch + 1) * 128],
                                         atc[:, t:t + w_],
                                         start=True, stop=True)
                        nc.scalar.activation(out=hT[:, cch, s0 + t:s0 + t + w_],
                                             in_=ps[:, 0:w_],
                                             func=mybir.ActivationFunctionType.Copy)
                    t += w_
                s0 += w_all

            def emit_proj(lw, a, pbuf, stfs, stfd, j):
                """Project block a with layer-lw weights into staging col j."""
                ps = psA.tile([128, 512], dt.float32, tag="projps")
                for kc in range(2):
                    nc.tensor.matmul(
                        ps[:],
                        hT[:, kc, a * 128:(a + 1) * 128],
                        wsd[:, lw * 2 + kc, :],
                        start=(kc == 0), stop=(kc == 1))
                nc.scalar.activation(out=stfs[:, j, :], in_=ps[:, 0:HID],
                                     func=mybir.ActivationFunctionType.Copy)
                nc.scalar.activation(out=stfd[:, j, :], in_=ps[:, HID:512],
                                     func=mybir.ActivationFunctionType.Copy)

            def emit_proj_dma(pbuf, blist, stfs, stfd):
                """DMA staged proj rows for blocks blist (consecutive or not)."""
                nb = len(blist)
                if blist == list(range(blist[0], blist[0] + nb)):
                    b0 = blist[0]
                    nc.sync.dma_start(
                        fs_bounce[pbuf][b0 * 128:(b0 + nb) * 128, :].rearrange(
                            "(a p) e -> p a e", p=128), stfs[:, 0:nb, :])
                    nc.sync.dma_start(
                        fd_tab[pbuf][b0 * 128:(b0 + nb) * 128, :].rearrange(
                            "(a p) e -> p a e", p=128), stfd[:, 0:nb, :])
                else:
                    for j, b in enumerate(blist):
                        nc.sync.dma_start(fs_bounce[pbuf][b * 128:(b + 1) * 128, :],
                                          stfs[:, j, :])
                        nc.sync.dma_start(fd_tab[pbuf][b * 128:(b + 1) * 128, :],
                                          stfd[:, j, :])

            # ---- layer-0 projection phase
            for a in range(NBE):
                j = a % 8
                if j == 0:
                    fs_sb = sp.tile([128, 8, HID], dt.bfloat16, tag="fs_sb")
                    fd_sb = sp.tile([128, 8, HID], dt.bfloat16, tag="fd_sb")
                    a0 = a
                emit_proj(0, a, 0, fs_sb, fd_sb, j)
                if j == 7 or a == NBE - 1:
                    emit_proj_dma(0, list(range(a0, a + 1)), fs_sb, fd_sb)

            for l in range(LAYERS):
                last = l == LAYERS - 1
                pbuf = l % 2
                nbuf = (l + 1) % 2

                # ---- AllGather fs table for this layer
                nc.gpsimd.collective_compute(
                    "AllGather", mybir.AluOpType.bypass,
                    replica_groups=[list(range(NCORES))],
                    ins=[fs_bounce[pbuf][:].opt()], outs=[fs_full[pbuf][:].opt()])

                # ---- per-superblock pipeline: [gather i][tail i-1][head i]
                def emit_gather(sb):
                    blocks, runs = sb['blocks'], sb['runs']
                    c0 = sb['col0']
                    fsg = gp.tile([128, CMAX, HID], dt.bfloat16, tag="fsg")
                    fd4 = fp.tile([128, SBMAXB, HID], dt.bfloat16, tag="fd4")
                    kruns = {}
                    for (k, b, R, g0) in runs:
                        lo, n = kruns.get(k, (g0, 0))
                        kruns[k] = (min(lo, g0), n + R)
                    for k, (g0, ncols_k) in sorted(kruns.items()):
                        lo = k * 2 * SLAB
                        hi = min(lo + 2 * SLAB, NCORES * SLAB)
                        for q0 in range(0, ncols_k, 8):
                            qn = min(8, ncols_k - q0)
                            g = g0 + q0
                            nc.gpsimd.dma_gather(
                                fsg[:, g - c0:g - c0 + qn, :],
                                fs_full[pbuf][lo:hi, :],
                                fsi[:, g * 8:(g + qn) * 8], qn * 128, qn * 128,
                                HID)
                    nb = len(blocks)
                    if blocks == list(range(blocks[0], blocks[0] + nb)):
                        b0 = blocks[0]
                        nc.sync.dma_start(
                            fd4[:, 0:nb, :],
                            fd_tab[pbuf][b0 * 128:(b0 + nb) * 128, :].rearrange(
                                "(a p) e -> p a e", p=128))
                    else:
                        for j, b in enumerate(blocks):
                            nc.sync.dma_start(fd4[:, j, :],
                                              fd_tab[pbuf][b * 128:(b + 1) * 128, :])
                    return fsg, fd4

                def emit_head(sb, fsg, fd4):
                    blocks, runs = sb['blocks'], sb['runs']
                    c0, nco = sb['col0'], sb['ncols']
                    u = up.tile([128, CMAX, HID], dt.bfloat16, tag="u")
                    for (k, b, R, g0) in runs:
                        j = blocks.index(b)
                        lc = g0 - c0
                        nc.vector.tensor_tensor(
                            out=u[:, lc:lc + R, :],
                            in0=fsg[:, lc:lc + R, :],
                            in1=fd4[:, j, :].unsqueeze(1).broadcast_to(
                                [128, R, HID]),
                            op=mybir.AluOpType.add)
                    nc.scalar.activation(out=u[:, 0:nco, :], in_=u[:, 0:nco, :],
                                         func=mybir.ActivationFunctionType.Prelu,
                                         alpha=ALPHA)
                    return u

                def emit_tail(sb, fsg, u):
                    blocks, runs = sb['blocks'], sb['runs']
                    c0, nco = sb['col0'], sb['ncols']
                    pn = cp.tile([128, CMAX, 8], dt.float32, tag="pn")
                    lg = cp.tile([128, CMAX, 4], dt.float32, tag="lg")
                    exb = cp.tile([128, CMAX, 4], dt.bfloat16, tag="exb")
                    m = cp.tile([128, CMAX, HID + 4], dt.bfloat16, tag="m")
                    for h in range(HEADS):
                        kp = int(pos_cnt[l, h])
                        kz = int(zero_cnt[l, h])
                        if kp > 0:
                            nc.vector.tensor_reduce(
                                out=pn[:, 0:nco, h],
                                in_=u[:, 0:nco, h * OUT:h * OUT + kp],
                                axis=mybir.AxisListType.X, op=mybir.AluOpType.add)
                        else:
                            nc.vector.memset(pn[:, 0:nco, h], 0.0)
                        if kp + kz < OUT:
                            nc.vector.tensor_reduce(
                                out=pn[:, 0:nco, 4 + h],
                                in_=u[:, 0:nco, h * OUT + kp + kz:(h + 1) * OUT],
                                axis=mybir.AxisListType.X, op=mybir.AluOpType.add)
                        else:
                            nc.vector.memset(pn[:, 0:nco, 4 + h], 0.0)
                    nc.vector.tensor_tensor(out=lg[:, 0:nco, :],
                                            in0=pn[:, 0:nco, 0:4],
                                            in1=pn[:, 0:nco, 4:8],
                                            op=mybir.AluOpType.subtract)
                    nc.vector.tensor_tensor(
                        out=lg[:, 0:nco, :], in0=lg[:, 0:nco, :],
                        in1=maskt[:, c0:c0 + nco].unsqueeze(2).broadcast_to(
                            [128, nco, 4]),
                        op=mybir.AluOpType.add)
                    nc.scalar.activation(out=exb[:, 0:nco, :],
                                         in_=lg[:, 0:nco, :],
                                         func=mybir.ActivationFunctionType.Exp)
                    nc.vector.tensor_tensor(
                        out=m[:, 0:nco, 0:HID].rearrange(
                            "p t (h d) -> p t h d", h=HEADS),
                        in0=fsg[:, 0:nco, :].rearrange(
                            "p t (h d) -> p t h d", h=HEADS),
                        in1=exb[:, 0:nco, :].unsqueeze(3).broadcast_to(
                            [128, nco, HEADS, OUT]),
                        op=mybir.AluOpType.mult)
                    nc.vector.tensor_copy(out=m[:, 0:nco, HID:HID + 4],
                                          in_=exb[:, 0:nco, :])

                    if not last:
                        prfs = sp.tile([128, SBMAXB, HID], dt.bfloat16, tag="fs_sb")
                        prfd = sp.tile([128, SBMAXB, HID], dt.bfloat16, tag="fd_sb")
                    nb = len(blocks)
                    rsd = cp.tile([128, SBMAXB, HID + 4], dt.bfloat16, tag="rsd")
                    for j, b in enumerate(blocks):
                        bruns = [(k, R, g0) for (k, bb, R, g0) in runs if bb == b]
                        cols = [g0 - c0 + r for (k, R, g0) in bruns
                                for r in range(R)]
                        if len(cols) == 1:
                            nc.vector.tensor_copy(out=rsd[:, j, :],
                                                  in_=m[:, cols[0], :])
                        else:
                            nc.vector.tensor_tensor(
                                out=rsd[:, j, :], in0=m[:, cols[0], :],
                                in1=m[:, cols[1], :], op=mybir.AluOpType.add)
                            for cc2 in cols[2:]:
                                nc.vector.tensor_tensor(
                                    out=rsd[:, j, :], in0=rsd[:, j, :],
                                    in1=m[:, cc2, :], op=mybir.AluOpType.add)
                    recs = cp.tile([128, SBMAXB, 4], dt.float32, tag="recs")
                    nc.vector.tensor_scalar(
                        out=recs[:, 0:nb, :], in0=rsd[:, 0:nb, HID:HID + 4],
                        scalar1=1e-30, scalar2=None, op0=mybir.AluOpType.max)
                    nc.vector.reciprocal(out=recs[:, 0:nb, :],
                                         in_=recs[:, 0:nb, :])
                    for j, b in enumerate(blocks):
                        hn = bp.tile([128, HID],
                                     dt.float32 if last else dt.bfloat16, tag="hn")
                        nc.vector.tensor_tensor(
                            out=hn[:].rearrange("p (h d) -> p h d", h=HEADS),
                            in0=rsd[:, j, 0:HID].rearrange(
                                "p (h d) -> p h d", h=HEADS),
                            in1=recs[:, j, :].unsqueeze(2).broadcast_to(
                                [128, HEADS, OUT]),
                            op=mybir.AluOpType.mult)
                        if last:
                            nc.sync.dma_start(out_d[b * 128:(b + 1) * 128, :],
                                              hn[:])
                        else:
                            for cch in range(2):
                                pt = psT.tile([128, 128], dt.bfloat16, tag="tp")
                                nc.tensor.transpose(
                                    pt[:], hn[:, cch * 128:(cch + 1) * 128],
                                    ident[:])
                                nc.scalar.activation(
                                    out=hT[:, cch, b * 128:(b + 1) * 128],
                                    in_=pt[:],
                                    func=mybir.ActivationFunctionType.Copy)
                            emit_proj(l + 1, b, nbuf, prfs, prfd, j)
                    if not last:
                        emit_proj_dma(nbuf, blocks, prfs, prfd)

                prev = None
                for sb in sbs:
                    fsg, fd4 = emit_gather(sb)
                    u = emit_head(sb, fsg, fd4)
                    if prev is not None:
                        emit_tail(*prev)
                    prev = (sb, fsg, u)
                emit_tail(*prev)

                # empty blocks: h = 0 for next layer, then project them
                if not last:
                    for i0 in range(0, len(empty_blocks), 8):
                        grp = empty_blocks[i0:i0 + 8]
                        efs = sp.tile([128, 8, HID], dt.bfloat16, tag="fs_sb")
                        efd = sp.tile([128, 8, HID], dt.bfloat16, tag="fd_sb")
                        for j, b in enumerate(grp):
                            nc.vector.memset(hT[:, 0, b * 128:(b + 1) * 128], 0.0)
                            nc.vector.memset(hT[:, 1, b * 128:(b + 1) * 128], 0.0)
                            emit_proj(l + 1, b, nbuf, efs, efd, j)
                        emit_proj_dma(nbuf, grp, efs, efd)
    nc.compile()
    return nc


def kernel(**inputs):
    from concourse.bass_utils import run_bass_kernel_spmd

    src = np.asarray(inputs['src'])
    dst = np.asarray(inputs['dst'])
    atom = np.asarray(inputs['atom_feat']).astype(np.float32)
    Ws_eff, Wd_eff, bs_eff, bd_eff, pos_cnt, zero_cnt, T2inv = _fold_weights(
        inputs['W_in'], inputs['b_in'], inputs['W_src'], inputs['b_src'],
        inputs['W_dst'], inputs['b_dst'], inputs['attn'], inputs['bias'])
    for l in range(LAYERS):
        assert np.abs(bs_eff[l]).max() < 1e-12 and np.abs(bd_eff[l]).max() < 1e-12, \
            "nonzero GAT biases not supported by this kernel build"

    Rtemp, assigns = build_structure(src, dst)
    slots = build_slots(src, dst, Rtemp, assigns)
    NBE, SLAB = slots['NBE'], slots['NBE'] * 128
    posmap = slots['posmap']

    win_np = np.zeros((ATOM_DIM + 1, HID), np.float32)
    win_np[:ATOM_DIM] = np.asarray(inputs['W_in'])
    win_np[ATOM_DIM] = np.asarray(inputs['b_in'])
    wsd_np = np.zeros((128, 2 * LAYERS, 512), np.float32)
    for l in range(LAYERS):
        for kc in range(2):
            wsd_np[:, l * 2 + kc, 0:HID] = Ws_eff[l][kc * 128:(kc + 1) * 128]
            wsd_np[:, l * 2 + kc, HID:512] = Wd_eff[l][kc * 128:(kc + 1) * 128]

    nc = _build(slots, pos_cnt, zero_cnt)

    ident = np.eye(128, dtype=np.float32)
    in_maps = []
    for c in range(NCORES):
        at = np.zeros((ATOM_DIM + 1, SLAB), np.float32)
        atc = atom[c * NPC:(c + 1) * NPC]
        at[:ATOM_DIM, posmap[c]] = atc.T
        at[ATOM_DIM, posmap[c]] = 1.0
        in_maps.append({
            'atomT': _bf(at), 'win': _bf(win_np), 'wsd': _bf(wsd_np),
            'fsi': _wrap16(slots['fsidx'][c]),
            'maskt': _bf(slots['mask'][c]),
            'ident': _bf(ident),
        })
    import os
    res = run_bass_kernel_spmd(nc, in_maps, core_ids=list(range(NCORES)),
                               trace=bool(os.environ.get('KBT_TRACE')))
    kernel._last = res
    out = np.zeros((N, HID), np.float64)
    for c in range(NCORES):
        out[c * NPC:(c + 1) * NPC] = res.results[c]['out'][posmap[c]]
    # zero rows for nodes in empty blocks (their h is exactly 0; device
    # never writes those rows)
    if slots['empty_blocks']:
        emptyset = np.zeros(SLAB, bool)
        for b in slots['empty_blocks']:
            emptyset[b * 128:(b + 1) * 128] = True
        for c in range(NCORES):
            zn = emptyset[posmap[c]]
            out[c * NPC:(c + 1) * NPC][zn] = 0.0
    out = out @ T2inv + np.asarray(inputs['bias'])[LAYERS - 1][None]
    return out.astype(np.float32)


if __name__ == '__main__':
    import jax
    with jax.default_device(jax.devices('cpu')[0]):
        import reference
        inputs = {k: np.asarray(v) for k, v in reference.setup_inputs().items()}
    got = kernel(**inputs)
    print("kernel out:", got.shape, got.dtype, np.abs(got).mean())


# revision 25
# speedup vs baseline: 1.0478x; 1.0478x over previous
"""Trainium2 8-core GATv2 message-passing kernel (nn_AtomGraphEncoder).

Dense (block, round, lane) design:
- Nodes sharded 8 x 12500 by id; WITHIN each core, nodes are permuted into
  blocks of 128 lanes with an SPMD-uniform per-class template R[b, k]
  (class k of an edge = src // 25000 = core-pair window, int16-gatherable).
- Per layer: proj fs/fd on PE (fd -> DRAM fd_tab, fs -> AllGather fs_full),
  then per superblock: one dma_gather per class window, dense free-dim ops:
  u = Prelu(fsg + fd bcast), sign-split reduces for GATv2 logits (|a| folded
  into W on host), exp (+pad mask -1e30), m = [ex*fs | ex], per-(b,k) strided
  reduce over rounds + per-block class-sum -> normalize -> hT / out.
- No scatter-add, no DRAM accumulators, no fd gather.
- Host: weight folding (undone on output), node permutation (undone on host).
"""
import sys

import numpy as np
import ml_dtypes

sys.path.insert(0, '/opt/trn_rl_repo')

N, E = 100000, 400000
ATOM_DIM, HID, LAYERS, HEADS = 74, 256, 3, 4
OUT = HID // HEADS
NCORES = 8
NPC = N // NCORES
W = 25000
NCLS = 4
BF = ml_dtypes.bfloat16
EPS = 1e-20
MASKNEG = -1e30


def _bf(x):
    return np.asarray(x).astype(BF)


def _fold_weights(W_in, b_in, W_src, b_src, W_dst, b_dst, attn, bias):
    Ts, Tinvs = [], []
    pos_cnt = np.zeros((LAYERS, HEADS), np.int64)
    zero_cnt = np.zeros((LAYERS, HEADS), np.int64)
    for l in range(LAYERS):
        Tl = np.zeros((HID, HID), np.float64)
        Tinv = np.zeros((HID, HID), np.float64)
        for h in range(HEADS):
            a = np.asarray(attn)[l, h].astype(np.float64)
            order = np.concatenate([
                np.where(a > 0)[0], np.where(a == 0)[0], np.where(a < 0)[0]])
            pos_cnt[l, h] = (a > 0).sum()
            zero_cnt[l, h] = (a == 0).sum()
            for j, p in enumerate(order):
                s = abs(a[p]) if a[p] != 0 else 1.0
                Tl[h * OUT + p, h * OUT + j] = s
                Tinv[h * OUT + j, h * OUT + p] = 1.0 / s
        Ts.append(Tl)
        Tinvs.append(Tinv)
    Ws_eff, Wd_eff, bs_eff, bd_eff = [], [], [], []
    for l in range(LAYERS):
        Tp = np.eye(HID) if l == 0 else Tinvs[l - 1]
        Ws = np.asarray(W_src)[l].astype(np.float64)
        Wd = np.asarray(W_dst)[l].astype(np.float64)
        bprev = np.zeros(HID) if l == 0 else np.asarray(bias)[l - 1].astype(np.float64)
        Ws_eff.append((Tp @ Ws @ Ts[l]).astype(np.float32))
        Wd_eff.append((Tp @ Wd @ Ts[l]).astype(np.float32))
        bs_eff.append(((np.asarray(b_src)[l] + bprev @ Ws) @ Ts[l]).astype(np.float32))
        bd_eff.append(((np.asarray(b_dst)[l] + bprev @ Wd) @ Ts[l]).astype(np.float32))
    return Ws_eff, Wd_eff, bs_eff, bd_eff, pos_cnt, zero_cnt, Tinvs[-1]


# ---------------------------------------------------------------- host prep
CMAX = 32           # max columns per superblock tile
SBMAXB = 8          # max blocks per superblock


def _percore_edges(src, dst):
    out = []
    for c in range(NCORES):
        m = (dst >= c * NPC) & (dst < (c + 1) * NPC)
        out.append((src[m], dst[m] - c * NPC))
    return out


def _class_counts(es, ed):
    cnt = np.zeros((NPC, NCLS), np.int64)
    np.add.at(cnt, (ed, es // W), 1)
    return cnt


def _lex_order(cnt):
    return np.lexsort((-cnt[:, 0], -cnt[:, 1], -cnt[:, 2], -cnt[:, 3]))


def _block_R(cnt, order, nb):
    cs = cnt[order]
    cs = np.vstack([cs, np.zeros((nb * 128 - len(cs), NCLS), np.int64)])
    return cs.reshape(nb, 128, NCLS).max(1)


def build_structure(src, dst, quant=0.45):
    src = np.asarray(src).astype(np.int64)
    dst = np.asarray(dst).astype(np.int64)
    percore = _percore_edges(src, dst)
    cnts = [_class_counts(es, ed) for es, ed in percore]
    NB = (NPC + 127) // 128

    Rs = np.stack([_block_R(cnts[c], _lex_order(cnts[c]), NB) for c in range(NCORES)])
    Rtemp = np.ceil(np.quantile(Rs, quant, axis=0)).astype(np.int64)

    assigns = np.full((NCORES, NB, 128), -1, np.int64)
    overflows = []
    for c in range(NCORES):
        cnt = cnts[c]
        order = _lex_order(cnt)
        cap = np.full(NB, 128, np.int64)
        fill = np.zeros(NB, np.int64)
        ov = []
        for idx in order:
            v = cnt[idx]
            placed = False
            for b in range(NB):
                if cap[b] > 0 and (Rtemp[b] >= v).all():
                    assigns[c, b, fill[b]] = idx
                    fill[b] += 1
                    cap[b] -= 1
                    placed = True
                    break
            if not placed:
                ov.append(idx)
        overflows.append(ov)

    nov_nodes = max(len(o) for o in overflows)
    n_ovb = (nov_nodes + 127) // 128
    if n_ovb:
        Rov = np.zeros((n_ovb, NCLS), np.int64)
        ovassign = np.full((NCORES, n_ovb, 128), -1, np.int64)
        for c in range(NCORES):
            ov = overflows[c]
            if ov:
                ovc = cnts[c][ov]
                o2 = np.lexsort((-ovc[:, 0], -ovc[:, 1], -ovc[:, 2], -ovc[:, 3]))
                ov = [ov[i] for i in o2]
            for i, idx in enumerate(ov):
                b = i // 128
                ovassign[c, b, i % 128] = idx
                np.maximum(Rov[b], cnts[c][idx], out=Rov[b])
        Rtemp = np.vstack([Rtemp, Rov])
        assigns = np.concatenate([assigns, ovassign], axis=1)
    return Rtemp, assigns


def build_slots(src, dst, Rtemp, assigns):
    src = np.asarray(src).astype(np.int64)
    dst = np.asarray(dst).astype(np.int64)
    NBE = Rtemp.shape[0]
    SLAB = NBE * 128

    posmap = np.full((NCORES, NPC), -1, np.int64)
    for c in range(NCORES):
        b_idx, lane_idx = np.nonzero(assigns[c] >= 0)
        nodes = assigns[c][b_idx, lane_idx]
        posmap[c, nodes] = b_idx * 128 + lane_idx

    core_of = np.arange(N) // NPC
    fsrow = core_of * SLAB + posmap[core_of, np.arange(N) % NPC]

    percore = _percore_edges(src, dst)

    Rsum = Rtemp.sum(1)
    sbs = []
    cur, cur_cols = [], 0
    for b in range(NBE):
        if Rsum[b] == 0:
            continue
        if cur and (cur_cols + Rsum[b] > CMAX or len(cur) == SBMAXB):
            sbs.append(cur)
            cur, cur_cols = [], 0
        cur.append(b)
        cur_cols += Rsum[b]
    if cur:
        sbs.append(cur)
    empty_blocks = [b for b in range(NBE) if Rsum[b] == 0]

    sb_meta = []
    gcol = 0
    for blocks in sbs:
        runs = []
        c0 = gcol
        for k in range(NCLS):
            for b in blocks:
                R = int(Rtemp[b, k])
                if R:
                    runs.append((k, b, R, gcol))
                    gcol += R
        sb_meta.append(dict(blocks=blocks, runs=runs, col0=c0, ncols=gcol - c0))
    C = gcol
    S = C * 128

    fsidx = np.zeros((NCORES, S), np.int64)
    mask = np.zeros((NCORES, 128, C), np.float32)
    for c in range(NCORES):
        es, ed = percore[c]
        cls = es // W
        order = np.lexsort((cls, ed))
        es_s, cls_s = es[order], cls[order]
        ed_s = ed[order]
        key = ed_s * NCLS + cls_s
        starts = np.searchsorted(key, np.arange(NPC * NCLS))
        ends = np.searchsorted(key, np.arange(NPC * NCLS), side='right')
        for sb in sb_meta:
            for (k, b, R, g0) in sb['runs']:
                for lane in range(128):
                    node = assigns[c, b, lane]
                    if node >= 0:
                        s0, s1 = starts[node * NCLS + k], ends[node * NCLS + k]
                        nn = min(R, s1 - s0)
                        for r in range(nn):
                            fsidx[c, (g0 + r) * 128 + lane] = \
                                fsrow[es_s[s0 + r]] - k * 2 * SLAB
                        for r in range(nn, R):
                            mask[c, lane, g0 + r] = MASKNEG
                    else:
                        for r in range(R):
                            mask[c, lane, g0 + r] = MASKNEG
    return dict(NBE=NBE, sbs=sb_meta, empty_blocks=empty_blocks, C=C, S=S,
                fsidx=fsidx, mask=mask, posmap=posmap)


def _wrap16(idx):
    w = np.ascontiguousarray(np.asarray(idx).reshape(-1, 16).T).astype(np.int16)
    return np.tile(w, (8, 1))


# ---------------------------------------------------------------- bass build
def _build(slots, pos_cnt, zero_cnt):
    import concourse.bass as bass
    import concourse.tile as tile
    from concourse import bacc, mybir, library_config

    NBE, sbs, C, S = slots['NBE'], slots['sbs'], slots['C'], slots['S']
    empty_blocks = slots['empty_blocks']
    SLAB = NBE * 128
    ALPHA = 0.2

    nc = bacc.Bacc("TRN2", target_bir_lowering=False, debug=False,
                   num_devices=NCORES)
    dt = mybir.dt
    atomT_d = nc.dram_tensor("atomT", [ATOM_DIM + 1, SLAB], dt.bfloat16,
                             kind="ExternalInput")
    win_d = nc.dram_tensor("win", [ATOM_DIM + 1, HID], dt.bfloat16,
                           kind="ExternalInput")
    wsd_d = nc.dram_tensor("wsd", [128, 2 * LAYERS, 512], dt.bfloat16,
                           kind="ExternalInput")
    fsi_d = nc.dram_tensor("fsi", [128, S // 16], dt.int16, kind="ExternalInput")
    mask_d = nc.dram_tensor("maskt", [128, C], dt.bfloat16, kind="ExternalInput")
    ident_d = nc.dram_tensor("ident", [128, 128], dt.bfloat16, kind="ExternalInput")
    out_d = nc.dram_tensor("out", [SLAB, HID], dt.float32, kind="ExternalOutput")

    fs_bounce = [nc.dram_tensor(f"fs_bounce{i}", [SLAB, HID], dt.bfloat16)
                 for i in range(2)]
    fd_tab = [nc.dram_tensor(f"fd_tab{i}", [SLAB, HID], dt.bfloat16)
              for i in range(2)]
    fs_full = [nc.dram_tensor(f"fs_full{i}", [NCORES * SLAB, HID], dt.bfloat16,
                              addr_space="Shared") for i in range(2)]

    with tile.TileContext(nc) as tc:
        nc.gpsimd.load_library(library_config.mlp)
        with tc.tile_pool(name="persist", bufs=1) as pp, \
             tc.tile_pool(name="gat", bufs=3) as gp, \
             tc.tile_pool(name="fdp", bufs=2) as fp, \
             tc.tile_pool(name="upool", bufs=2) as up, \
             tc.tile_pool(name="cmp", bufs=1) as cp, \
             tc.tile_pool(name="blk", bufs=2) as bp, \
             tc.tile_pool(name="stage", bufs=2) as sp, \
             tc.tile_pool(name="psA", bufs=2, space="PSUM") as psA, \
             tc.tile_pool(name="psT", bufs=2, space="PSUM") as psT:

            fsi = pp.tile([128, S // 16], dt.int16, tag="fsi")
            maskt = pp.tile([128, C], dt.bfloat16, tag="maskt")
            wsd = pp.tile([128, 2 * LAYERS, 512], dt.bfloat16, tag="wsd")
            win = pp.tile([ATOM_DIM + 1, HID], dt.bfloat16, tag="win")
            ident = pp.tile([128, 128], dt.bfloat16, tag="ident")
            hT = pp.tile([128, 2, SLAB], dt.bfloat16, tag="hT")
            nc.sync.dma_start(fsi[:], fsi_d[:])
            nc.sync.dma_start(maskt[:], mask_d[:])
            nc.sync.dma_start(wsd[:], wsd_d[:])
            nc.sync.dma_start(win[:], win_d[:])
            nc.sync.dma_start(ident[:], ident_d[:])

            # ---- input projection: hT0 = (atomT^T @ Win)^T (chunk-streamed)
            CH = 1024
            s0 = 0
            while s0 < SLAB:
                w_all = min(CH, SLAB - s0)
                atc = sp.tile([ATOM_DIM + 1, CH], dt.bfloat16, tag="atc")
                nc.sync.dma_start(atc[:, 0:w_all], atomT_d[:, s0:s0 + w_all])
                t = 0
                while t < w_all:
                    w_ = min(512, w_all - t)
                    for cch in range(2):
                        ps = psA.tile([128, 512], dt.float32, tag="projps")
                        nc.tensor.matmul(ps[:, 0:w_],
                                         win[:, cch * 128:(cch + 1) * 128],
                                         atc[:, t:t + w_],
                                         start=True, stop=True)
                        nc.scalar.activation(out=hT[:, cch, s0 + t:s0 + t + w_],
                                             in_=ps[:, 0:w_],
                                             func=mybir.ActivationFunctionType.Copy)
                    t += w_
                s0 += w_all

            def emit_proj(lw, a, pbuf, stfs, stfd, j):
                """Project block a with layer-lw weights into staging col j."""
                ps = psA.tile([128, 512], dt.float32, tag="projps")
                for kc in range(2):
                    nc.tensor.matmul(
                        ps[:],
                        hT[:, kc, a * 128:(a + 1) * 128],
                        wsd[:, lw * 2 + kc, :],
                        start=(kc == 0), stop=(kc == 1))
                nc.scalar.activation(out=stfs[:, j, :], in_=ps[:, 0:HID],
                                     func=mybir.ActivationFunctionType.Copy)
                nc.scalar.activation(out=stfd[:, j, :], in_=ps[:, HID:512],
                                     func=mybir.ActivationFunctionType.Copy)

            def emit_proj_dma(pbuf, blist, stfs, stfd):
                """DMA staged proj rows for blocks blist (consecutive or not)."""
                nb = len(blist)
                if blist == list(range(blist[0], blist[0] + nb)):
                    b0 = blist[0]
                    nc.sync.dma_start(
                        fs_bounce[pbuf][b0 * 128:(b0 + nb) * 128, :].rearrange(
                            "(a p) e -> p a e", p=128), stfs[:, 0:nb, :])
                    nc.sync.dma_start(
                        fd_tab[pbuf][b0 * 128:(b0 + nb) * 128, :].rearrange(
                            "(a p) e -> p a e", p=128), stfd[:, 0:nb, :])
                else:
                    for j, b in enumerate(blist):
                        nc.sync.dma_start(fs_bounce[pbuf][b * 128:(b + 1) * 128, :],
                                          stfs[:, j, :])
                        nc.sync.dma_start(fd_tab[pbuf][b * 128:(b + 1) * 128, :],
                                          stfd[:, j, :])

            # ---- layer-0 projection phase
            for a in range(NBE):
                j = a % 8
                if j == 0:
                    fs_sb = sp.tile([128, 8, HID], dt.bfloat16, tag="fs_sb")
                    fd_sb = sp.tile([128, 8, HID], dt.bfloat16, tag="fd_sb")
                    a0 = a
                emit_proj(0, a, 0, fs_sb, fd_sb, j)
                if j == 7 or a == NBE - 1:
                    emit_proj_dma(0, list(range(a0, a + 1)), fs_sb, fd_sb)

            for l in range(LAYERS):
                last = l == LAYERS - 1
                pbuf = l % 2
                nbuf = (l + 1) % 2

                # ---- AllGather fs table for this layer
                nc.gpsimd.collective_compute(
                    "AllGather", mybir.AluOpType.bypass,
                    replica_groups=[list(range(NCORES))],
                    ins=[fs_bounce[pbuf][:].opt()], outs=[fs_full[pbuf][:].opt()])

                # ---- per-superblock pipeline: [gather i][tail i-1][head i]
                def emit_gather(sb):
                    blocks, runs = sb['blocks'], sb['runs']
                    c0 = sb['col0']
                    fsg = gp.tile([128, CMAX, HID], dt.bfloat16, tag="fsg")
                    fd4 = fp.tile([128, SBMAXB, HID], dt.bfloat16, tag="fd4")
                    kruns = {}
                    for (k, b, R, g0) in runs:
                        lo, n = kruns.get(k, (g0, 0))
                        kruns[k] = (min(lo, g0), n + R)
                    for k, (g0, ncols_k) in sorted(kruns.items()):
                        lo = k * 2 * SLAB
                        hi = min(lo + 2 * SLAB, NCORES * SLAB)
                        for q0 in range(0, ncols_k, 8):
                            qn = min(8, ncols_k - q0)
                            g = g0 + q0
                            nc.gpsimd.dma_gather(
                                fsg[:, g - c0:g - c0 + qn, :],
                                fs_full[pbuf][lo:hi, :],
                                fsi[:, g * 8:(g + qn) * 8], qn * 128, qn * 128,
                                HID)
                    nb = len(blocks)
                    if blocks == list(range(blocks[0], blocks[0] + nb)):
                        b0 = blocks[0]
                        nc.sync.dma_start(
                            fd4[:, 0:nb, :],
                            fd_tab[pbuf][b0 * 128:(b0 + nb) * 128, :].rearrange(
                                "(a p) e -> p a e", p=128))
                    else:
                        for j, b in enumerate(blocks):
                            nc.sync.dma_start(fd4[:, j, :],
                                              fd_tab[pbuf][b * 128:(b + 1) * 128, :])
                    return fsg, fd4

                def emit_head(sb, fsg, fd4):
                    blocks, runs = sb['blocks'], sb['runs']
                    c0, nco = sb['col0'], sb['ncols']
                    u = up.tile([128, CMAX, HID], dt.bfloat16, tag="u")
                    for (k, b, R, g0) in runs:
                        j = blocks.index(b)
                        lc = g0 - c0
                        nc.vector.tensor_tensor(
                            out=u[:, lc:lc + R, :],
                            in0=fsg[:, lc:lc + R, :],
                            in1=fd4[:, j, :].unsqueeze(1).broadcast_to(
                                [128, R, HID]),
                            op=mybir.AluOpType.add)
                    nc.scalar.activation(out=u[:, 0:nco, :], in_=u[:, 0:nco, :],
                                         func=mybir.ActivationFunctionType.Prelu,
                                         alpha=ALPHA)
                    return u

                def emit_tailA(sb, fsg, u):
                    blocks, runs = sb['blocks'], sb['runs']
                    c0, nco = sb['col0'], sb['ncols']
                    pn = cp.tile([128, CMAX, 8], dt.float32, tag="pn")
                    lg = cp.tile([128, CMAX, 4], dt.float32, tag="lg")
                    exb = cp.tile([128, CMAX, 4], dt.bfloat16, tag="exb")
                    for h in range(HEADS):
                        kp = int(pos_cnt[l, h])
                        kz = int(zero_cnt[l, h])
                        if kp > 0:
                            nc.vector.tensor_reduce(
                                out=pn[:, 0:nco, h],
                                in_=u[:, 0:nco, h * OUT:h * OUT + kp],
                                axis=mybir.AxisListType.X, op=mybir.AluOpType.add)
                        else:
                            nc.vector.memset(pn[:, 0:nco, h], 0.0)
                        if kp + kz < OUT:
                            nc.vector.tensor_reduce(
                                out=pn[:, 0:nco, 4 + h],
                                in_=u[:, 0:nco, h * OUT + kp + kz:(h + 1) * OUT],
                                axis=mybir.AxisListType.X, op=mybir.AluOpType.add)
                        else:
                            nc.vector.memset(pn[:, 0:nco, 4 + h], 0.0)
                    nc.vector.tensor_tensor(out=lg[:, 0:nco, :],
                                            in0=pn[:, 0:nco, 0:4],
                                            in1=pn[:, 0:nco, 4:8],
                                            op=mybir.AluOpType.subtract)
                    nc.vector.tensor_tensor(
                        out=lg[:, 0:nco, :], in0=lg[:, 0:nco, :],
                        in1=maskt[:, c0:c0 + nco].unsqueeze(2).broadcast_to(
                            [128, nco, 4]),
                        op=mybir.AluOpType.add)
                    nc.scalar.activation(out=exb[:, 0:nco, :],
                                         in_=lg[:, 0:nco, :],
                                         func=mybir.ActivationFunctionType.Exp)
                    return exb

                def emit_tailB(sb, fsg, u, exb):
                    blocks, runs = sb['blocks'], sb['runs']
                    c0, nco = sb['col0'], sb['ncols']
                    m = cp.tile([128, CMAX, HID + 4], dt.bfloat16, tag="m")
                    nc.vector.tensor_tensor(
                        out=m[:, 0:nco, 0:HID].rearrange(
                            "p t (h d) -> p t h d", h=HEADS),
                        in0=fsg[:, 0:nco, :].rearrange(
                            "p t (h d) -> p t h d", h=HEADS),
                        in1=exb[:, 0:nco, :].unsqueeze(3).broadcast_to(
                            [128, nco, HEADS, OUT]),
                        op=mybir.AluOpType.mult)
                    nc.vector.tensor_copy(out=m[:, 0:nco, HID:HID + 4],
                                          in_=exb[:, 0:nco, :])

                    if not last:
                        prfs = sp.tile([128, SBMAXB, HID], dt.bfloat16, tag="fs_sb")
                        prfd = sp.tile([128, SBMAXB, HID], dt.bfloat16, tag="fd_sb")
                    nb = len(blocks)
                    rsd = cp.tile([128, SBMAXB, HID + 4], dt.bfloat16, tag="rsd")
                    for j, b in enumerate(blocks):
                        bruns = [(k, R, g0) for (k, bb, R, g0) in runs if bb == b]
                        cols = [g0 - c0 + r for (k, R, g0) in bruns
                                for r in range(R)]
                        if len(cols) == 1:
                            nc.vector.tensor_copy(out=rsd[:, j, :],
                                                  in_=m[:, cols[0], :])
                        else:
                            nc.vector.tensor_tensor(
                                out=rsd[:, j, :], in0=m[:, cols[0], :],
                                in1=m[:, cols[1], :], op=mybir.AluOpType.add)
                            for cc2 in cols[2:]:
                                nc.vector.tensor_tensor(
                                    out=rsd[:, j, :], in0=rsd[:, j, :],
                                    in1=m[:, cc2, :], op=mybir.AluOpType.add)
                    recs = cp.tile([128, SBMAXB, 4], dt.float32, tag="recs")
                    nc.vector.tensor_scalar(
                        out=recs[:, 0:nb, :], in0=rsd[:, 0:nb, HID:HID + 4],
                        scalar1=1e-30, scalar2=None, op0=mybir.AluOpType.max)
                    nc.vector.reciprocal(out=recs[:, 0:nb, :],
                                         in_=recs[:, 0:nb, :])
                    for j, b in enumerate(blocks):
                        hn = bp.tile([128, HID],
                                     dt.float32 if last else dt.bfloat16, tag="hn")
                        nc.vector.tensor_tensor(
                            out=hn[:].rearrange("p (h d) -> p h d", h=HEADS),
                            in0=rsd[:, j, 0:HID].rearrange(
                                "p (h d) -> p h d", h=HEADS),
                            in1=recs[:, j, :].unsqueeze(2).broadcast_to(
                                [128, HEADS, OUT]),
                            op=mybir.AluOpType.mult)
                        if last:
                            nc.sync.dma_start(out_d[b * 128:(b + 1) * 128, :],
                                              hn[:])
                        else:
                            for cch in range(2):
                                pt = psT.tile([128, 128], dt.bfloat16, tag="tp")
                                nc.tensor.transpose(
                                    pt[:], hn[:, cch * 128:(cch + 1) * 128],
                                    ident[:])
                                nc.scalar.activation(
                                    out=hT[:, cch, b * 128:(b + 1) * 128],
                                    in_=pt[:],
                                    func=mybir.ActivationFunctionType.Copy)
                            emit_proj(l + 1, b, nbuf, prfs, prfd, j)
                    if not last:
                        emit_proj_dma(nbuf, blocks, prfs, prfd)

                prev = None
                for sb in sbs:
                    fsg, fd4 = emit_gather(sb)
                    if prev is not None:
                        exb = emit_tailA(*prev)
                    u = emit_head(sb, fsg, fd4)
                    if prev is not None:
                        emit_tailB(*prev, exb)
                    prev = (sb, fsg, u)
                exb = emit_tailA(*prev)
                emit_tailB(*prev, exb)

                # empty blocks: h = 0 for next layer, then project them
                if not last:
                    for i0 in range(0, len(empty_blocks), 8):
                        grp = empty_blocks[i0:i0 + 8]
                        efs = sp.tile([128, 8, HID], dt.bfloat16, tag="fs_sb")
                        efd = sp.tile([128, 8, HID], dt.bfloat16, tag="fd_sb")
                        for j, b in enumerate(grp):
                            nc.vector.memset(hT[:, 0, b * 128:(b + 1) * 128], 0.0)
                            nc.vector.memset(hT[:, 1, b * 128:(b + 1) * 128], 0.0)
                            emit_proj(l + 1, b, nbuf, efs, efd, j)
                        emit_proj_dma(nbuf, grp, efs, efd)
    nc.compile()
    return nc


def kernel(**inputs):
    from concourse.bass_utils import run_bass_kernel_spmd

    src = np.asarray(inputs['src'])
    dst = np.asarray(inputs['dst'])
    atom = np.asarray(inputs['atom_feat']).astype(np.float32)
    Ws_eff, Wd_eff, bs_eff, bd_eff, pos_cnt, zero_cnt, T2inv = _fold_weights(
        inputs['W_in'], inputs['b_in'], inputs['W_src'], inputs['b_src'],
        inputs['W_dst'], inputs['b_dst'], inputs['attn'], inputs['bias'])
    for l in range(LAYERS):
        assert np.abs(bs_eff[l]).max() < 1e-12 and np.abs(bd_eff[l]).max() < 1e-12, \
            "nonzero GAT biases not supported by this kernel build"

    Rtemp, assigns = build_structure(src, dst)
    slots = build_slots(src, dst, Rtemp, assigns)
    NBE, SLAB = slots['NBE'], slots['NBE'] * 128
    posmap = slots['posmap']

    win_np = np.zeros((ATOM_DIM + 1, HID), np.float32)
    win_np[:ATOM_DIM] = np.asarray(inputs['W_in'])
    win_np[ATOM_DIM] = np.asarray(inputs['b_in'])
    wsd_np = np.zeros((128, 2 * LAYERS, 512), np.float32)
    for l in range(LAYERS):
        for kc in range(2):
            wsd_np[:, l * 2 + kc, 0:HID] = Ws_eff[l][kc * 128:(kc + 1) * 128]
            wsd_np[:, l * 2 + kc, HID:512] = Wd_eff[l][kc * 128:(kc + 1) * 128]

    nc = _build(slots, pos_cnt, zero_cnt)

    ident = np.eye(128, dtype=np.float32)
    in_maps = []
    for c in range(NCORES):
        at = np.zeros((ATOM_DIM + 1, SLAB), np.float32)
        atc = atom[c * NPC:(c + 1) * NPC]
        at[:ATOM_DIM, posmap[c]] = atc.T
        at[ATOM_DIM, posmap[c]] = 1.0
        in_maps.append({
            'atomT': _bf(at), 'win': _bf(win_np), 'wsd': _bf(wsd_np),
            'fsi': _wrap16(slots['fsidx'][c]),
            'maskt': _bf(slots['mask'][c]),
            'ident': _bf(ident),
        })
    import os
    res = run_bass_kernel_spmd(nc, in_maps, core_ids=list(range(NCORES)),
                               trace=bool(os.environ.get('KBT_TRACE')))
    kernel._last = res
    out = np.zeros((N, HID), np.float64)
    for c in range(NCORES):
        out[c * NPC:(c + 1) * NPC] = res.results[c]['out'][posmap[c]]
    # zero rows for nodes in empty blocks (their h is exactly 0; device
    # never writes those rows)
    if slots['empty_blocks']:
        emptyset = np.zeros(SLAB, bool)
        for b in slots['empty_blocks']:
            emptyset[b * 128:(b + 1) * 128] = True
        for c in range(NCORES):
            zn = emptyset[posmap[c]]
            out[c * NPC:(c + 1) * NPC][zn] = 0.0
    out = out @ T2inv + np.asarray(inputs['bias'])[LAYERS - 1][None]
    return out.astype(np.float32)


if __name__ == '__main__':
    import jax
    with jax.default_device(jax.devices('cpu')[0]):
        import reference
        inputs = {k: np.asarray(v) for k, v in reference.setup_inputs().items()}
    got = kernel(**inputs)
    print("kernel out:", got.shape, got.dtype, np.abs(got).mean())


# revision 26
# speedup vs baseline: 1.0509x; 1.0030x over previous
"""Trainium2 8-core GATv2 message-passing kernel (nn_AtomGraphEncoder).

Dense (block, round, lane) design:
- Nodes sharded 8 x 12500 by id; WITHIN each core, nodes are permuted into
  blocks of 128 lanes with an SPMD-uniform per-class template R[b, k]
  (class k of an edge = src // 25000 = core-pair window, int16-gatherable).
- Per layer: proj fs/fd on PE (fd -> DRAM fd_tab, fs -> AllGather fs_full),
  then per superblock: one dma_gather per class window, dense free-dim ops:
  u = Prelu(fsg + fd bcast), sign-split reduces for GATv2 logits (|a| folded
  into W on host), exp (+pad mask -1e30), m = [ex*fs | ex], per-(b,k) strided
  reduce over rounds + per-block class-sum -> normalize -> hT / out.
- No scatter-add, no DRAM accumulators, no fd gather.
- Host: weight folding (undone on output), node permutation (undone on host).
"""
import sys

import numpy as np
import ml_dtypes

sys.path.insert(0, '/opt/trn_rl_repo')

N, E = 100000, 400000
ATOM_DIM, HID, LAYERS, HEADS = 74, 256, 3, 4
OUT = HID // HEADS
NCORES = 8
NPC = N // NCORES
W = 25000
NCLS = 4
BF = ml_dtypes.bfloat16
EPS = 1e-20
MASKNEG = -1e30


def _bf(x):
    return np.asarray(x).astype(BF)


def _fold_weights(W_in, b_in, W_src, b_src, W_dst, b_dst, attn, bias):
    Ts, Tinvs = [], []
    pos_cnt = np.zeros((LAYERS, HEADS), np.int64)
    zero_cnt = np.zeros((LAYERS, HEADS), np.int64)
    for l in range(LAYERS):
        Tl = np.zeros((HID, HID), np.float64)
        Tinv = np.zeros((HID, HID), np.float64)
        for h in range(HEADS):
            a = np.asarray(attn)[l, h].astype(np.float64)
            order = np.concatenate([
                np.where(a > 0)[0], np.where(a == 0)[0], np.where(a < 0)[0]])
            pos_cnt[l, h] = (a > 0).sum()
            zero_cnt[l, h] = (a == 0).sum()
            for j, p in enumerate(order):
                s = abs(a[p]) if a[p] != 0 else 1.0
                Tl[h * OUT + p, h * OUT + j] = s
                Tinv[h * OUT + j, h * OUT + p] = 1.0 / s
        Ts.append(Tl)
        Tinvs.append(Tinv)
    Ws_eff, Wd_eff, bs_eff, bd_eff = [], [], [], []
    for l in range(LAYERS):
        Tp = np.eye(HID) if l == 0 else Tinvs[l - 1]
        Ws = np.asarray(W_src)[l].astype(np.float64)
        Wd = np.asarray(W_dst)[l].astype(np.float64)
        bprev = np.zeros(HID) if l == 0 else np.asarray(bias)[l - 1].astype(np.float64)
        Ws_eff.append((Tp @ Ws @ Ts[l]).astype(np.float32))
        Wd_eff.append((Tp @ Wd @ Ts[l]).astype(np.float32))
        bs_eff.append(((np.asarray(b_src)[l] + bprev @ Ws) @ Ts[l]).astype(np.float32))
        bd_eff.append(((np.asarray(b_dst)[l] + bprev @ Wd) @ Ts[l]).astype(np.float32))
    return Ws_eff, Wd_eff, bs_eff, bd_eff, pos_cnt, zero_cnt, Tinvs[-1]


# ---------------------------------------------------------------- host prep
CMAX = 32           # max columns per superblock tile
SBMAXB = 8          # max blocks per superblock


def _percore_edges(src, dst):
    out = []
    for c in range(NCORES):
        m = (dst >= c * NPC) & (dst < (c + 1) * NPC)
        out.append((src[m], dst[m] - c * NPC))
    return out


def _class_counts(es, ed):
    cnt = np.zeros((NPC, NCLS), np.int64)
    np.add.at(cnt, (ed, es // W), 1)
    return cnt


def _lex_order(cnt):
    return np.lexsort((-cnt[:, 0], -cnt[:, 1], -cnt[:, 2], -cnt[:, 3]))


def _block_R(cnt, order, nb):
    cs = cnt[order]
    cs = np.vstack([cs, np.zeros((nb * 128 - len(cs), NCLS), np.int64)])
    return cs.reshape(nb, 128, NCLS).max(1)


def build_structure(src, dst, quant=0.45):
    src = np.asarray(src).astype(np.int64)
    dst = np.asarray(dst).astype(np.int64)
    percore = _percore_edges(src, dst)
    cnts = [_class_counts(es, ed) for es, ed in percore]
    NB = (NPC + 127) // 128

    Rs = np.stack([_block_R(cnts[c], _lex_order(cnts[c]), NB) for c in range(NCORES)])
    Rtemp = np.ceil(np.quantile(Rs, quant, axis=0)).astype(np.int64)

    assigns = np.full((NCORES, NB, 128), -1, np.int64)
    overflows = []
    for c in range(NCORES):
        cnt = cnts[c]
        order = _lex_order(cnt)
        cap = np.full(NB, 128, np.int64)
        fill = np.zeros(NB, np.int64)
        ov = []
        for idx in order:
            v = cnt[idx]
            placed = False
            for b in range(NB):
                if cap[b] > 0 and (Rtemp[b] >= v).all():
                    assigns[c, b, fill[b]] = idx
                    fill[b] += 1
                    cap[b] -= 1
                    placed = True
                    break
            if not placed:
                ov.append(idx)
        overflows.append(ov)

    nov_nodes = max(len(o) for o in overflows)
    n_ovb = (nov_nodes + 127) // 128
    if n_ovb:
        Rov = np.zeros((n_ovb, NCLS), np.int64)
        ovassign = np.full((NCORES, n_ovb, 128), -1, np.int64)
        for c in range(NCORES):
            ov = overflows[c]
            if ov:
                ovc = cnts[c][ov]
                o2 = np.lexsort((-ovc[:, 0], -ovc[:, 1], -ovc[:, 2], -ovc[:, 3]))
                ov = [ov[i] for i in o2]
            for i, idx in enumerate(ov):
                b = i // 128
                ovassign[c, b, i % 128] = idx
                np.maximum(Rov[b], cnts[c][idx], out=Rov[b])
        Rtemp = np.vstack([Rtemp, Rov])
        assigns = np.concatenate([assigns, ovassign], axis=1)
    return Rtemp, assigns


def build_slots(src, dst, Rtemp, assigns):
    src = np.asarray(src).astype(np.int64)
    dst = np.asarray(dst).astype(np.int64)
    NBE = Rtemp.shape[0]
    SLAB = NBE * 128

    posmap = np.full((NCORES, NPC), -1, np.int64)
    for c in range(NCORES):
        b_idx, lane_idx = np.nonzero(assigns[c] >= 0)
        nodes = assigns[c][b_idx, lane_idx]
        posmap[c, nodes] = b_idx * 128 + lane_idx

    core_of = np.arange(N) // NPC
    fsrow = core_of * SLAB + posmap[core_of, np.arange(N) % NPC]

    percore = _percore_edges(src, dst)

    Rsum = Rtemp.sum(1)
    sbs = []
    cur, cur_cols = [], 0
    for b in range(NBE):
        if Rsum[b] == 0:
            continue
        if cur and (cur_cols + Rsum[b] > CMAX or len(cur) == SBMAXB):
            sbs.append(cur)
            cur, cur_cols = [], 0
        cur.append(b)
        cur_cols += Rsum[b]
    if cur:
        sbs.append(cur)
    empty_blocks = [b for b in range(NBE) if Rsum[b] == 0]

    sb_meta = []
    gcol = 0
    for blocks in sbs:
        runs = []
        c0 = gcol
        for k in range(NCLS):
            for b in blocks:
                R = int(Rtemp[b, k])
                if R:
                    runs.append((k, b, R, gcol))
                    gcol += R
        sb_meta.append(dict(blocks=blocks, runs=runs, col0=c0, ncols=gcol - c0))
    C = gcol
    S = C * 128

    fsidx = np.zeros((NCORES, S), np.int64)
    mask = np.zeros((NCORES, 128, C), np.float32)
    for c in range(NCORES):
        es, ed = percore[c]
        cls = es // W
        order = np.lexsort((cls, ed))
        es_s, cls_s = es[order], cls[order]
        ed_s = ed[order]
        key = ed_s * NCLS + cls_s
        starts = np.searchsorted(key, np.arange(NPC * NCLS))
        ends = np.searchsorted(key, np.arange(NPC * NCLS), side='right')
        for sb in sb_meta:
            for (k, b, R, g0) in sb['runs']:
                for lane in range(128):
                    node = assigns[c, b, lane]
                    if node >= 0:
                        s0, s1 = starts[node * NCLS + k], ends[node * NCLS + k]
                        nn = min(R, s1 - s0)
                        for r in range(nn):
                            fsidx[c, (g0 + r) * 128 + lane] = \
                                fsrow[es_s[s0 + r]] - k * 2 * SLAB
                        for r in range(nn, R):
                            mask[c, lane, g0 + r] = MASKNEG
                    else:
                        for r in range(R):
                            mask[c, lane, g0 + r] = MASKNEG
    return dict(NBE=NBE, sbs=sb_meta, empty_blocks=empty_blocks, C=C, S=S,
                fsidx=fsidx, mask=mask, posmap=posmap)


def _wrap16(idx):
    w = np.ascontiguousarray(np.asarray(idx).reshape(-1, 16).T).astype(np.int16)
    return np.tile(w, (8, 1))


# ---------------------------------------------------------------- bass build
def _build(slots, pos_cnt, zero_cnt):
    import concourse.bass as bass
    import concourse.tile as tile
    from concourse import bacc, mybir, library_config

    NBE, sbs, C, S = slots['NBE'], slots['sbs'], slots['C'], slots['S']
    empty_blocks = slots['empty_blocks']
    SLAB = NBE * 128
    ALPHA = 0.2

    nc = bacc.Bacc("TRN2", target_bir_lowering=False, debug=False,
                   num_devices=NCORES)
    dt = mybir.dt
    atomT_d = nc.dram_tensor("atomT", [ATOM_DIM + 1, SLAB], dt.bfloat16,
                             kind="ExternalInput")
    win_d = nc.dram_tensor("win", [ATOM_DIM + 1, HID], dt.bfloat16,
                           kind="ExternalInput")
    wsd_d = nc.dram_tensor("wsd", [128, 2 * LAYERS, 512], dt.bfloat16,
                           kind="ExternalInput")
    fsi_d = nc.dram_tensor("fsi", [128, S // 16], dt.int16, kind="ExternalInput")
    mask_d = nc.dram_tensor("maskt", [128, C], dt.bfloat16, kind="ExternalInput")
    ident_d = nc.dram_tensor("ident", [128, 128], dt.bfloat16, kind="ExternalInput")
    out_d = nc.dram_tensor("out", [SLAB, HID], dt.float32, kind="ExternalOutput")

    fs_bounce = [nc.dram_tensor(f"fs_bounce{i}", [SLAB, HID], dt.bfloat16)
                 for i in range(2)]
    fd_tab = [nc.dram_tensor(f"fd_tab{i}", [SLAB, HID], dt.bfloat16)
              for i in range(2)]
    fs_full = [nc.dram_tensor(f"fs_full{i}", [NCORES * SLAB, HID], dt.bfloat16,
                              addr_space="Shared") for i in range(2)]

    with tile.TileContext(nc) as tc:
        nc.gpsimd.load_library(library_config.mlp)
        with tc.tile_pool(name="persist", bufs=1) as pp, \
             tc.tile_pool(name="gat", bufs=3) as gp, \
             tc.tile_pool(name="fdp", bufs=2) as fp, \
             tc.tile_pool(name="upool", bufs=2) as up, \
             tc.tile_pool(name="cmp", bufs=1) as cp, \
             tc.tile_pool(name="blk", bufs=2) as bp, \
             tc.tile_pool(name="stage", bufs=2) as sp, \
             tc.tile_pool(name="psA", bufs=2, space="PSUM") as psA, \
             tc.tile_pool(name="psT", bufs=2, space="PSUM") as psT:

            fsi = pp.tile([128, S // 16], dt.int16, tag="fsi")
            maskt = pp.tile([128, C], dt.bfloat16, tag="maskt")
            wsd = pp.tile([128, 2 * LAYERS, 512], dt.bfloat16, tag="wsd")
            win = pp.tile([ATOM_DIM + 1, HID], dt.bfloat16, tag="win")
            ident = pp.tile([128, 128], dt.bfloat16, tag="ident")
            hT = pp.tile([128, 2, SLAB], dt.bfloat16, tag="hT")
            nc.sync.dma_start(fsi[:], fsi_d[:])
            nc.sync.dma_start(maskt[:], mask_d[:])
            nc.sync.dma_start(wsd[:], wsd_d[:])
            nc.sync.dma_start(win[:], win_d[:])
            nc.sync.dma_start(ident[:], ident_d[:])

            # ---- input projection: hT0 = (atomT^T @ Win)^T (chunk-streamed)
            CH = 1024
            s0 = 0
            while s0 < SLAB:
                w_all = min(CH, SLAB - s0)
                atc = sp.tile([ATOM_DIM + 1, CH], dt.bfloat16, tag="atc")
                nc.sync.dma_start(atc[:, 0:w_all], atomT_d[:, s0:s0 + w_all])
                t = 0
                while t < w_all:
                    w_ = min(512, w_all - t)
                    for cch in range(2):
                        ps = psA.tile([128, 512], dt.float32, tag="projps")
                        nc.tensor.matmul(ps[:, 0:w_],
                                         win[:, cch * 128:(cch + 1) * 128],
                                         atc[:, t:t + w_],
                                         start=True, stop=True)
                        nc.scalar.activation(out=hT[:, cch, s0 + t:s0 + t + w_],
                                             in_=ps[:, 0:w_],
                                             func=mybir.ActivationFunctionType.Copy)
                    t += w_
                s0 += w_all

            def emit_proj(lw, a, pbuf, stfs, stfd, j):
                """Project block a with layer-lw weights into staging col j."""
                ps = psA.tile([128, 512], dt.float32, tag="projps")
                for kc in range(2):
                    nc.tensor.matmul(
                        ps[:],
                        hT[:, kc, a * 128:(a + 1) * 128],
                        wsd[:, lw * 2 + kc, :],
                        start=(kc == 0), stop=(kc == 1))
                nc.scalar.activation(out=stfs[:, j, :], in_=ps[:, 0:HID],
                                     func=mybir.ActivationFunctionType.Copy)
                nc.scalar.activation(out=stfd[:, j, :], in_=ps[:, HID:512],
                                     func=mybir.ActivationFunctionType.Copy)

            def emit_proj_dma(pbuf, blist, stfs, stfd):
                """DMA staged proj rows for blocks blist (consecutive or not)."""
                nb = len(blist)
                if blist == list(range(blist[0], blist[0] + nb)):
                    b0 = blist[0]
                    nc.sync.dma_start(
                        fs_bounce[pbuf][b0 * 128:(b0 + nb) * 128, :].rearrange(
                            "(a p) e -> p a e", p=128), stfs[:, 0:nb, :])
                    nc.sync.dma_start(
                        fd_tab[pbuf][b0 * 128:(b0 + nb) * 128, :].rearrange(
                            "(a p) e -> p a e", p=128), stfd[:, 0:nb, :])
                else:
                    for j, b in enumerate(blist):
                        nc.sync.dma_start(fs_bounce[pbuf][b * 128:(b + 1) * 128, :],
                                          stfs[:, j, :])
                        nc.sync.dma_start(fd_tab[pbuf][b * 128:(b + 1) * 128, :],
                                          stfd[:, j, :])

            # ---- layer-0 projection phase
            for a in range(NBE):
                j = a % 8
                if j == 0:
                    fs_sb = sp.tile([128, 8, HID], dt.bfloat16, tag="fs_sb")
                    fd_sb = sp.tile([128, 8, HID], dt.bfloat16, tag="fd_sb")
                    a0 = a
                emit_proj(0, a, 0, fs_sb, fd_sb, j)
                if j == 7 or a == NBE - 1:
                    emit_proj_dma(0, list(range(a0, a + 1)), fs_sb, fd_sb)

            for l in range(LAYERS):
                last = l == LAYERS - 1
                pbuf = l % 2
                nbuf = (l + 1) % 2

                # ---- AllGather fs table for this layer
                nc.gpsimd.collective_compute(
                    "AllGather", mybir.AluOpType.bypass,
                    replica_groups=[list(range(NCORES))],
                    ins=[fs_bounce[pbuf][:].opt()], outs=[fs_full[pbuf][:].opt()])

                # ---- per-superblock pipeline: [gather i][tail i-1][head i]
                def emit_gather(sb):
                    blocks, runs = sb['blocks'], sb['runs']
                    c0 = sb['col0']
                    fsg = gp.tile([128, CMAX, HID], dt.bfloat16, tag="fsg")
                    fd4 = fp.tile([128, SBMAXB, HID], dt.bfloat16, tag="fd4")
                    kruns = {}
                    for (k, b, R, g0) in runs:
                        lo, n = kruns.get(k, (g0, 0))
                        kruns[k] = (min(lo, g0), n + R)
                    for k, (g0, ncols_k) in sorted(kruns.items()):
                        lo = k * 2 * SLAB
                        hi = min(lo + 2 * SLAB, NCORES * SLAB)
                        for q0 in range(0, ncols_k, 16):
                            qn = min(16, ncols_k - q0)
                            g = g0 + q0
                            nc.gpsimd.dma_gather(
                                fsg[:, g - c0:g - c0 + qn, :],
                                fs_full[pbuf][lo:hi, :],
                                fsi[:, g * 8:(g + qn) * 8], qn * 128, qn * 128,
                                HID, single_packet=False)
                    nb = len(blocks)
                    if blocks == list(range(blocks[0], blocks[0] + nb)):
                        b0 = blocks[0]
                        nc.sync.dma_start(
                            fd4[:, 0:nb, :],
                            fd_tab[pbuf][b0 * 128:(b0 + nb) * 128, :].rearrange(
                                "(a p) e -> p a e", p=128))
                    else:
                        for j, b in enumerate(blocks):
                            nc.sync.dma_start(fd4[:, j, :],
                                              fd_tab[pbuf][b * 128:(b + 1) * 128, :])
                    return fsg, fd4

                def emit_head(sb, fsg, fd4):
                    blocks, runs = sb['blocks'], sb['runs']
                    c0, nco = sb['col0'], sb['ncols']
                    u = up.tile([128, CMAX, HID], dt.bfloat16, tag="u")
                    for (k, b, R, g0) in runs:
                        j = blocks.index(b)
                        lc = g0 - c0
                        nc.vector.tensor_tensor(
                            out=u[:, lc:lc + R, :],
                            in0=fsg[:, lc:lc + R, :],
                            in1=fd4[:, j, :].unsqueeze(1).broadcast_to(
                                [128, R, HID]),
                            op=mybir.AluOpType.add)
                    nc.scalar.activation(out=u[:, 0:nco, :], in_=u[:, 0:nco, :],
                                         func=mybir.ActivationFunctionType.Prelu,
                                         alpha=ALPHA)
                    return u

                def emit_tailA(sb, fsg, u):
                    blocks, runs = sb['blocks'], sb['runs']
                    c0, nco = sb['col0'], sb['ncols']
                    pn = cp.tile([128, CMAX, 8], dt.float32, tag="pn")
                    lg = cp.tile([128, CMAX, 4], dt.float32, tag="lg")
                    exb = cp.tile([128, CMAX, 4], dt.bfloat16, tag="exb")
                    for h in range(HEADS):
                        kp = int(pos_cnt[l, h])
                        kz = int(zero_cnt[l, h])
                        if kp > 0:
                            nc.vector.tensor_reduce(
                                out=pn[:, 0:nco, h],
                                in_=u[:, 0:nco, h * OUT:h * OUT + kp],
                                axis=mybir.AxisListType.X, op=mybir.AluOpType.add)
                        else:
                            nc.vector.memset(pn[:, 0:nco, h], 0.0)
                        if kp + kz < OUT:
                            nc.vector.tensor_reduce(
                                out=pn[:, 0:nco, 4 + h],
                                in_=u[:, 0:nco, h * OUT + kp + kz:(h + 1) * OUT],
                                axis=mybir.AxisListType.X, op=mybir.AluOpType.add)
                        else:
                            nc.vector.memset(pn[:, 0:nco, 4 + h], 0.0)
                    nc.vector.tensor_tensor(out=lg[:, 0:nco, :],
                                            in0=pn[:, 0:nco, 0:4],
                                            in1=pn[:, 0:nco, 4:8],
                                            op=mybir.AluOpType.subtract)
                    nc.vector.tensor_tensor(
                        out=lg[:, 0:nco, :], in0=lg[:, 0:nco, :],
                        in1=maskt[:, c0:c0 + nco].unsqueeze(2).broadcast_to(
                            [128, nco, 4]),
                        op=mybir.AluOpType.add)
                    nc.scalar.activation(out=exb[:, 0:nco, :],
                                         in_=lg[:, 0:nco, :],
                                         func=mybir.ActivationFunctionType.Exp)
                    return exb

                def emit_tailB(sb, fsg, u, exb):
                    blocks, runs = sb['blocks'], sb['runs']
                    c0, nco = sb['col0'], sb['ncols']
                    m = cp.tile([128, CMAX, HID + 4], dt.bfloat16, tag="m")
                    nc.vector.tensor_tensor(
                        out=m[:, 0:nco, 0:HID].rearrange(
                            "p t (h d) -> p t h d", h=HEADS),
                        in0=fsg[:, 0:nco, :].rearrange(
                            "p t (h d) -> p t h d", h=HEADS),
                        in1=exb[:, 0:nco, :].unsqueeze(3).broadcast_to(
                            [128, nco, HEADS, OUT]),
                        op=mybir.AluOpType.mult)
                    nc.vector.tensor_copy(out=m[:, 0:nco, HID:HID + 4],
                                          in_=exb[:, 0:nco, :])

                    if not last:
                        prfs = sp.tile([128, SBMAXB, HID], dt.bfloat16, tag="fs_sb")
                        prfd = sp.tile([128, SBMAXB, HID], dt.bfloat16, tag="fd_sb")
                    nb = len(blocks)
                    rsd = cp.tile([128, SBMAXB, HID + 4], dt.bfloat16, tag="rsd")
                    for j, b in enumerate(blocks):
                        bruns = [(k, R, g0) for (k, bb, R, g0) in runs if bb == b]
                        cols = [g0 - c0 + r for (k, R, g0) in bruns
                                for r in range(R)]
                        if len(cols) == 1:
                            nc.vector.tensor_copy(out=rsd[:, j, :],
                                                  in_=m[:, cols[0], :])
                        else:
                            nc.vector.tensor_tensor(
                                out=rsd[:, j, :], in0=m[:, cols[0], :],
                                in1=m[:, cols[1], :], op=mybir.AluOpType.add)
                            for cc2 in cols[2:]:
                                nc.vector.tensor_tensor(
                                    out=rsd[:, j, :], in0=rsd[:, j, :],
                                    in1=m[:, cc2, :], op=mybir.AluOpType.add)
                    recs = cp.tile([128, SBMAXB, 4], dt.float32, tag="recs")
                    nc.vector.tensor_scalar(
                        out=recs[:, 0:nb, :], in0=rsd[:, 0:nb, HID:HID + 4],
                        scalar1=1e-30, scalar2=None, op0=mybir.AluOpType.max)
                    nc.vector.reciprocal(out=recs[:, 0:nb, :],
                                         in_=recs[:, 0:nb, :])
                    for j, b in enumerate(blocks):
                        hn = bp.tile([128, HID],
                                     dt.float32 if last else dt.bfloat16, tag="hn")
                        nc.vector.tensor_tensor(
                            out=hn[:].rearrange("p (h d) -> p h d", h=HEADS),
                            in0=rsd[:, j, 0:HID].rearrange(
                                "p (h d) -> p h d", h=HEADS),
                            in1=recs[:, j, :].unsqueeze(2).broadcast_to(
                                [128, HEADS, OUT]),
                            op=mybir.AluOpType.mult)
                        if last:
                            nc.sync.dma_start(out_d[b * 128:(b + 1) * 128, :],
                                              hn[:])
                        else:
                            for cch in range(2):
                                pt = psT.tile([128, 128], dt.bfloat16, tag="tp")
                                nc.tensor.transpose(
                                    pt[:], hn[:, cch * 128:(cch + 1) * 128],
                                    ident[:])
                                nc.scalar.activation(
                                    out=hT[:, cch, b * 128:(b + 1) * 128],
                                    in_=pt[:],
                                    func=mybir.ActivationFunctionType.Copy)
                            emit_proj(l + 1, b, nbuf, prfs, prfd, j)
                    if not last:
                        emit_proj_dma(nbuf, blocks, prfs, prfd)

                prev = None
                for sb in sbs:
                    fsg, fd4 = emit_gather(sb)
                    if prev is not None:
                        exb = emit_tailA(*prev)
                    u = emit_head(sb, fsg, fd4)
                    if prev is not None:
                        emit_tailB(*prev, exb)
                    prev = (sb, fsg, u)
                exb = emit_tailA(*prev)
                emit_tailB(*prev, exb)

                # empty blocks: h = 0 for next layer, then project them
                if not last:
                    for i0 in range(0, len(empty_blocks), 8):
                        grp = empty_blocks[i0:i0 + 8]
                        efs = sp.tile([128, 8, HID], dt.bfloat16, tag="fs_sb")
                        efd = sp.tile([128, 8, HID], dt.bfloat16, tag="fd_sb")
                        for j, b in enumerate(grp):
                            nc.vector.memset(hT[:, 0, b * 128:(b + 1) * 128], 0.0)
                            nc.vector.memset(hT[:, 1, b * 128:(b + 1) * 128], 0.0)
                            emit_proj(l + 1, b, nbuf, efs, efd, j)
                        emit_proj_dma(nbuf, grp, efs, efd)
    nc.compile()
    return nc


def kernel(**inputs):
    from concourse.bass_utils import run_bass_kernel_spmd

    src = np.asarray(inputs['src'])
    dst = np.asarray(inputs['dst'])
    atom = np.asarray(inputs['atom_feat']).astype(np.float32)
    Ws_eff, Wd_eff, bs_eff, bd_eff, pos_cnt, zero_cnt, T2inv = _fold_weights(
        inputs['W_in'], inputs['b_in'], inputs['W_src'], inputs['b_src'],
        inputs['W_dst'], inputs['b_dst'], inputs['attn'], inputs['bias'])
    for l in range(LAYERS):
        assert np.abs(bs_eff[l]).max() < 1e-12 and np.abs(bd_eff[l]).max() < 1e-12, \
            "nonzero GAT biases not supported by this kernel build"

    Rtemp, assigns = build_structure(src, dst)
    slots = build_slots(src, dst, Rtemp, assigns)
    NBE, SLAB = slots['NBE'], slots['NBE'] * 128
    posmap = slots['posmap']

    win_np = np.zeros((ATOM_DIM + 1, HID), np.float32)
    win_np[:ATOM_DIM] = np.asarray(inputs['W_in'])
    win_np[ATOM_DIM] = np.asarray(inputs['b_in'])
    wsd_np = np.zeros((128, 2 * LAYERS, 512), np.float32)
    for l in range(LAYERS):
        for kc in range(2):
            wsd_np[:, l * 2 + kc, 0:HID] = Ws_eff[l][kc * 128:(kc + 1) * 128]
            wsd_np[:, l * 2 + kc, HID:512] = Wd_eff[l][kc * 128:(kc + 1) * 128]

    nc = _build(slots, pos_cnt, zero_cnt)

    ident = np.eye(128, dtype=np.float32)
    in_maps = []
    for c in range(NCORES):
        at = np.zeros((ATOM_DIM + 1, SLAB), np.float32)
        atc = atom[c * NPC:(c + 1) * NPC]
        at[:ATOM_DIM, posmap[c]] = atc.T
        at[ATOM_DIM, posmap[c]] = 1.0
        in_maps.append({
            'atomT': _bf(at), 'win': _bf(win_np), 'wsd': _bf(wsd_np),
            'fsi': _wrap16(slots['fsidx'][c]),
            'maskt': _bf(slots['mask'][c]),
            'ident': _bf(ident),
        })
    import os
    res = run_bass_kernel_spmd(nc, in_maps, core_ids=list(range(NCORES)),
                               trace=bool(os.environ.get('KBT_TRACE')))
    kernel._last = res
    out = np.zeros((N, HID), np.float64)
    for c in range(NCORES):
        out[c * NPC:(c + 1) * NPC] = res.results[c]['out'][posmap[c]]
    # zero rows for nodes in empty blocks (their h is exactly 0; device
    # never writes those rows)
    if slots['empty_blocks']:
        emptyset = np.zeros(SLAB, bool)
        for b in slots['empty_blocks']:
            emptyset[b * 128:(b + 1) * 128] = True
        for c in range(NCORES):
            zn = emptyset[posmap[c]]
            out[c * NPC:(c + 1) * NPC][zn] = 0.0
    out = out @ T2inv + np.asarray(inputs['bias'])[LAYERS - 1][None]
    return out.astype(np.float32)


if __name__ == '__main__':
    import jax
    with jax.default_device(jax.devices('cpu')[0]):
        import reference
        inputs = {k: np.asarray(v) for k, v in reference.setup_inputs().items()}
    got = kernel(**inputs)
    print("kernel out:", got.shape, got.dtype, np.abs(got).mean())


# revision 27
# speedup vs baseline: 1.0868x; 1.0342x over previous
"""Trainium2 8-core GATv2 message-passing kernel (nn_AtomGraphEncoder).

Dense (block, round, lane) design:
- Nodes sharded 8 x 12500 by id; WITHIN each core, nodes are permuted into
  blocks of 128 lanes with an SPMD-uniform per-class template R[b, k]
  (class k of an edge = src // 25000 = core-pair window, int16-gatherable).
- Per layer: proj fs/fd on PE (fd -> DRAM fd_tab, fs -> AllGather fs_full),
  then per superblock: one dma_gather per class window, dense free-dim ops:
  u = Prelu(fsg + fd bcast), sign-split reduces for GATv2 logits (|a| folded
  into W on host), exp (+pad mask -1e30), m = [ex*fs | ex], per-(b,k) strided
  reduce over rounds + per-block class-sum -> normalize -> hT / out.
- No scatter-add, no DRAM accumulators, no fd gather.
- Host: weight folding (undone on output), node permutation (undone on host).
"""
import sys

import numpy as np
import ml_dtypes

sys.path.insert(0, '/opt/trn_rl_repo')

N, E = 100000, 400000
ATOM_DIM, HID, LAYERS, HEADS = 74, 256, 3, 4
OUT = HID // HEADS
NCORES = 8
NPC = N // NCORES
W = 25000
NCLS = 4
BF = ml_dtypes.bfloat16
EPS = 1e-20
MASKNEG = -1e30


def _bf(x):
    return np.asarray(x).astype(BF)


def _fold_weights(W_in, b_in, W_src, b_src, W_dst, b_dst, attn, bias):
    Ts, Tinvs = [], []
    pos_cnt = np.zeros((LAYERS, HEADS), np.int64)
    zero_cnt = np.zeros((LAYERS, HEADS), np.int64)
    for l in range(LAYERS):
        Tl = np.zeros((HID, HID), np.float64)
        Tinv = np.zeros((HID, HID), np.float64)
        for h in range(HEADS):
            a = np.asarray(attn)[l, h].astype(np.float64)
            order = np.concatenate([
                np.where(a > 0)[0], np.where(a == 0)[0], np.where(a < 0)[0]])
            pos_cnt[l, h] = (a > 0).sum()
            zero_cnt[l, h] = (a == 0).sum()
            for j, p in enumerate(order):
                s = abs(a[p]) if a[p] != 0 else 1.0
                Tl[h * OUT + p, h * OUT + j] = s
                Tinv[h * OUT + j, h * OUT + p] = 1.0 / s
        Ts.append(Tl)
        Tinvs.append(Tinv)
    Ws_eff, Wd_eff, bs_eff, bd_eff = [], [], [], []
    for l in range(LAYERS):
        Tp = np.eye(HID) if l == 0 else Tinvs[l - 1]
        Ws = np.asarray(W_src)[l].astype(np.float64)
        Wd = np.asarray(W_dst)[l].astype(np.float64)
        bprev = np.zeros(HID) if l == 0 else np.asarray(bias)[l - 1].astype(np.float64)
        Ws_eff.append((Tp @ Ws @ Ts[l]).astype(np.float32))
        Wd_eff.append((Tp @ Wd @ Ts[l]).astype(np.float32))
        bs_eff.append(((np.asarray(b_src)[l] + bprev @ Ws) @ Ts[l]).astype(np.float32))
        bd_eff.append(((np.asarray(b_dst)[l] + bprev @ Wd) @ Ts[l]).astype(np.float32))
    return Ws_eff, Wd_eff, bs_eff, bd_eff, pos_cnt, zero_cnt, Tinvs[-1]


# ---------------------------------------------------------------- host prep
CMAX = 32           # max columns per superblock tile
SBMAXB = 8          # max blocks per superblock


def _percore_edges(src, dst):
    out = []
    for c in range(NCORES):
        m = (dst >= c * NPC) & (dst < (c + 1) * NPC)
        out.append((src[m], dst[m] - c * NPC))
    return out


def _class_counts(es, ed):
    cnt = np.zeros((NPC, NCLS), np.int64)
    np.add.at(cnt, (ed, es // W), 1)
    return cnt


def _lex_order(cnt):
    return np.lexsort((-cnt[:, 0], -cnt[:, 1], -cnt[:, 2], -cnt[:, 3]))


def _block_R(cnt, order, nb):
    cs = cnt[order]
    cs = np.vstack([cs, np.zeros((nb * 128 - len(cs), NCLS), np.int64)])
    return cs.reshape(nb, 128, NCLS).max(1)


def build_structure(src, dst, quant=0.40):
    src = np.asarray(src).astype(np.int64)
    dst = np.asarray(dst).astype(np.int64)
    percore = _percore_edges(src, dst)
    cnts = [_class_counts(es, ed) for es, ed in percore]
    NB = (NPC + 127) // 128

    Rs = np.stack([_block_R(cnts[c], _lex_order(cnts[c]), NB) for c in range(NCORES)])
    Rtemp = np.ceil(np.quantile(Rs, quant, axis=0)).astype(np.int64)

    assigns = np.full((NCORES, NB, 128), -1, np.int64)
    overflows = []
    for c in range(NCORES):
        cnt = cnts[c]
        order = _lex_order(cnt)
        cap = np.full(NB, 128, np.int64)
        fill = np.zeros(NB, np.int64)
        ov = []
        for idx in order:
            v = cnt[idx]
            placed = False
            for b in range(NB):
                if cap[b] > 0 and (Rtemp[b] >= v).all():
                    assigns[c, b, fill[b]] = idx
                    fill[b] += 1
                    cap[b] -= 1
                    placed = True
                    break
            if not placed:
                ov.append(idx)
        overflows.append(ov)

    nov_nodes = max(len(o) for o in overflows)
    n_ovb = (nov_nodes + 127) // 128
    if n_ovb:
        Rov = np.zeros((n_ovb, NCLS), np.int64)
        ovassign = np.full((NCORES, n_ovb, 128), -1, np.int64)
        for c in range(NCORES):
            ov = overflows[c]
            if ov:
                ovc = cnts[c][ov]
                o2 = np.lexsort((-ovc[:, 0], -ovc[:, 1], -ovc[:, 2], -ovc[:, 3]))
                ov = [ov[i] for i in o2]
            for i, idx in enumerate(ov):
                b = i // 128
                ovassign[c, b, i % 128] = idx
                np.maximum(Rov[b], cnts[c][idx], out=Rov[b])
        Rtemp = np.vstack([Rtemp, Rov])
        assigns = np.concatenate([assigns, ovassign], axis=1)
    return Rtemp, assigns


def build_slots(src, dst, Rtemp, assigns):
    src = np.asarray(src).astype(np.int64)
    dst = np.asarray(dst).astype(np.int64)
    NBE = Rtemp.shape[0]
    SLAB = NBE * 128

    posmap = np.full((NCORES, NPC), -1, np.int64)
    for c in range(NCORES):
        b_idx, lane_idx = np.nonzero(assigns[c] >= 0)
        nodes = assigns[c][b_idx, lane_idx]
        posmap[c, nodes] = b_idx * 128 + lane_idx

    core_of = np.arange(N) // NPC
    fsrow = core_of * SLAB + posmap[core_of, np.arange(N) % NPC]

    percore = _percore_edges(src, dst)

    Rsum = Rtemp.sum(1)
    sbs = []
    cur, cur_cols = [], 0
    for b in range(NBE):
        if Rsum[b] == 0:
            continue
        if cur and (cur_cols + Rsum[b] > CMAX or len(cur) == SBMAXB):
            sbs.append(cur)
            cur, cur_cols = [], 0
        cur.append(b)
        cur_cols += Rsum[b]
    if cur:
        sbs.append(cur)
    empty_blocks = [b for b in range(NBE) if Rsum[b] == 0]

    sb_meta = []
    gcol = 0
    for blocks in sbs:
        runs = []
        c0 = gcol
        for k in range(NCLS):
            for b in blocks:
                R = int(Rtemp[b, k])
                if R:
                    runs.append((k, b, R, gcol))
                    gcol += R
        sb_meta.append(dict(blocks=blocks, runs=runs, col0=c0, ncols=gcol - c0))
    C = gcol
    S = C * 128

    fsidx = np.zeros((NCORES, S), np.int64)
    mask = np.zeros((NCORES, 128, C), np.float32)
    for c in range(NCORES):
        es, ed = percore[c]
        cls = es // W
        order = np.lexsort((cls, ed))
        es_s, cls_s = es[order], cls[order]
        ed_s = ed[order]
        key = ed_s * NCLS + cls_s
        starts = np.searchsorted(key, np.arange(NPC * NCLS))
        ends = np.searchsorted(key, np.arange(NPC * NCLS), side='right')
        for sb in sb_meta:
            for (k, b, R, g0) in sb['runs']:
                for lane in range(128):
                    node = assigns[c, b, lane]
                    if node >= 0:
                        s0, s1 = starts[node * NCLS + k], ends[node * NCLS + k]
                        nn = min(R, s1 - s0)
                        for r in range(nn):
                            fsidx[c, (g0 + r) * 128 + lane] = \
                                fsrow[es_s[s0 + r]] - k * 2 * SLAB
                        for r in range(nn, R):
                            mask[c, lane, g0 + r] = MASKNEG
                    else:
                        for r in range(R):
                            mask[c, lane, g0 + r] = MASKNEG
    return dict(NBE=NBE, sbs=sb_meta, empty_blocks=empty_blocks, C=C, S=S,
                fsidx=fsidx, mask=mask, posmap=posmap)


def _wrap16(idx):
    w = np.ascontiguousarray(np.asarray(idx).reshape(-1, 16).T).astype(np.int16)
    return np.tile(w, (8, 1))


# ---------------------------------------------------------------- bass build
def _build(slots, pos_cnt, zero_cnt):
    import concourse.bass as bass
    import concourse.tile as tile
    from concourse import bacc, mybir, library_config

    NBE, sbs, C, S = slots['NBE'], slots['sbs'], slots['C'], slots['S']
    empty_blocks = slots['empty_blocks']
    SLAB = NBE * 128
    ALPHA = 0.2

    nc = bacc.Bacc("TRN2", target_bir_lowering=False, debug=False,
                   num_devices=NCORES)
    dt = mybir.dt
    atomT_d = nc.dram_tensor("atomT", [ATOM_DIM + 1, SLAB], dt.bfloat16,
                             kind="ExternalInput")
    win_d = nc.dram_tensor("win", [ATOM_DIM + 1, HID], dt.bfloat16,
                           kind="ExternalInput")
    wsd_d = nc.dram_tensor("wsd", [128, 2 * LAYERS, 512], dt.bfloat16,
                           kind="ExternalInput")
    fsi_d = nc.dram_tensor("fsi", [128, S // 16], dt.int16, kind="ExternalInput")
    mask_d = nc.dram_tensor("maskt", [128, C], dt.bfloat16, kind="ExternalInput")
    ident_d = nc.dram_tensor("ident", [128, 128], dt.bfloat16, kind="ExternalInput")
    out_d = nc.dram_tensor("out", [SLAB, HID], dt.float32, kind="ExternalOutput")

    fs_bounce = [nc.dram_tensor(f"fs_bounce{i}", [SLAB, HID], dt.bfloat16)
                 for i in range(2)]
    fd_tab = [nc.dram_tensor(f"fd_tab{i}", [SLAB, HID], dt.bfloat16)
              for i in range(2)]
    fs_full = [nc.dram_tensor(f"fs_full{i}", [NCORES * SLAB, HID], dt.bfloat16,
                              addr_space="Shared") for i in range(2)]

    with tile.TileContext(nc) as tc:
        nc.gpsimd.load_library(library_config.mlp)
        with tc.tile_pool(name="persist", bufs=1) as pp, \
             tc.tile_pool(name="gat", bufs=3) as gp, \
             tc.tile_pool(name="fdp", bufs=2) as fp, \
             tc.tile_pool(name="upool", bufs=2) as up, \
             tc.tile_pool(name="cmp", bufs=1) as cp, \
             tc.tile_pool(name="blk", bufs=2) as bp, \
             tc.tile_pool(name="stage", bufs=2) as sp, \
             tc.tile_pool(name="psA", bufs=2, space="PSUM") as psA, \
             tc.tile_pool(name="psT", bufs=2, space="PSUM") as psT:

            fsi = pp.tile([128, S // 16], dt.int16, tag="fsi")
            maskt = pp.tile([128, C], dt.bfloat16, tag="maskt")
            wsd = pp.tile([128, 2 * LAYERS, 512], dt.bfloat16, tag="wsd")
            win = pp.tile([ATOM_DIM + 1, HID], dt.bfloat16, tag="win")
            ident = pp.tile([128, 128], dt.bfloat16, tag="ident")
            hT = pp.tile([128, 2, SLAB], dt.bfloat16, tag="hT")
            nc.sync.dma_start(fsi[:], fsi_d[:])
            nc.sync.dma_start(maskt[:], mask_d[:])
            nc.sync.dma_start(wsd[:], wsd_d[:])
            nc.sync.dma_start(win[:], win_d[:])
            nc.sync.dma_start(ident[:], ident_d[:])

            def emit_proj(lw, a, pbuf, stfs, stfd, j):
                """Project block a with layer-lw weights into staging col j."""
                ps = psA.tile([128, 512], dt.float32, tag="projps")
                for kc in range(2):
                    nc.tensor.matmul(
                        ps[:],
                        hT[:, kc, a * 128:(a + 1) * 128],
                        wsd[:, lw * 2 + kc, :],
                        start=(kc == 0), stop=(kc == 1))
                nc.scalar.activation(out=stfs[:, j, :], in_=ps[:, 0:HID],
                                     func=mybir.ActivationFunctionType.Copy)
                nc.scalar.activation(out=stfd[:, j, :], in_=ps[:, HID:512],
                                     func=mybir.ActivationFunctionType.Copy)

            def emit_proj_dma(pbuf, blist, stfs, stfd):
                """DMA staged proj rows for blocks blist (consecutive or not)."""
                nb = len(blist)
                if blist == list(range(blist[0], blist[0] + nb)):
                    b0 = blist[0]
                    nc.sync.dma_start(
                        fs_bounce[pbuf][b0 * 128:(b0 + nb) * 128, :].rearrange(
                            "(a p) e -> p a e", p=128), stfs[:, 0:nb, :])
                    nc.sync.dma_start(
                        fd_tab[pbuf][b0 * 128:(b0 + nb) * 128, :].rearrange(
                            "(a p) e -> p a e", p=128), stfd[:, 0:nb, :])
                else:
                    for j, b in enumerate(blist):
                        nc.sync.dma_start(fs_bounce[pbuf][b * 128:(b + 1) * 128, :],
                                          stfs[:, j, :])
                        nc.sync.dma_start(fd_tab[pbuf][b * 128:(b + 1) * 128, :],
                                          stfd[:, j, :])

            # ---- input projection fused with layer-0 projection per chunk
            CH = 1024
            s0 = 0
            while s0 < SLAB:
                w_all = min(CH, SLAB - s0)
                atc = sp.tile([ATOM_DIM + 1, CH], dt.bfloat16, tag="atc")
                nc.sync.dma_start(atc[:, 0:w_all], atomT_d[:, s0:s0 + w_all])
                t = 0
                while t < w_all:
                    w_ = min(512, w_all - t)
                    for cch in range(2):
                        ps = psA.tile([128, 512], dt.float32, tag="projps")
                        nc.tensor.matmul(ps[:, 0:w_],
                                         win[:, cch * 128:(cch + 1) * 128],
                                         atc[:, t:t + w_],
                                         start=True, stop=True)
                        nc.scalar.activation(out=hT[:, cch, s0 + t:s0 + t + w_],
                                             in_=ps[:, 0:w_],
                                             func=mybir.ActivationFunctionType.Copy)
                    t += w_
                nblk = w_all // 128
                b0 = s0 // 128
                fs_sb = sp.tile([128, 8, HID], dt.bfloat16, tag="fs_sb")
                fd_sb = sp.tile([128, 8, HID], dt.bfloat16, tag="fd_sb")
                for j in range(nblk):
                    emit_proj(0, b0 + j, 0, fs_sb, fd_sb, j)
                emit_proj_dma(0, list(range(b0, b0 + nblk)), fs_sb, fd_sb)
                s0 += w_all

            for l in range(LAYERS):
                last = l == LAYERS - 1
                pbuf = l % 2
                nbuf = (l + 1) % 2

                # ---- AllGather fs table for this layer
                nc.gpsimd.collective_compute(
                    "AllGather", mybir.AluOpType.bypass,
                    replica_groups=[list(range(NCORES))],
                    ins=[fs_bounce[pbuf][:].opt()], outs=[fs_full[pbuf][:].opt()])

                # ---- per-superblock pipeline: [gather i][tail i-1][head i]
                def emit_gather(sb):
                    blocks, runs = sb['blocks'], sb['runs']
                    c0 = sb['col0']
                    fsg = gp.tile([128, CMAX, HID], dt.bfloat16, tag="fsg")
                    fd4 = fp.tile([128, SBMAXB, HID], dt.bfloat16, tag="fd4")
                    kruns = {}
                    for (k, b, R, g0) in runs:
                        lo, n = kruns.get(k, (g0, 0))
                        kruns[k] = (min(lo, g0), n + R)
                    for k, (g0, ncols_k) in sorted(kruns.items()):
                        lo = k * 2 * SLAB
                        hi = min(lo + 2 * SLAB, NCORES * SLAB)
                        for q0 in range(0, ncols_k, 16):
                            qn = min(16, ncols_k - q0)
                            g = g0 + q0
                            nc.gpsimd.dma_gather(
                                fsg[:, g - c0:g - c0 + qn, :],
                                fs_full[pbuf][lo:hi, :],
                                fsi[:, g * 8:(g + qn) * 8], qn * 128, qn * 128,
                                HID, single_packet=False)
                    nb = len(blocks)
                    if blocks == list(range(blocks[0], blocks[0] + nb)):
                        b0 = blocks[0]
                        nc.sync.dma_start(
                            fd4[:, 0:nb, :],
                            fd_tab[pbuf][b0 * 128:(b0 + nb) * 128, :].rearrange(
                                "(a p) e -> p a e", p=128))
                    else:
                        for j, b in enumerate(blocks):
                            nc.sync.dma_start(fd4[:, j, :],
                                              fd_tab[pbuf][b * 128:(b + 1) * 128, :])
                    return fsg, fd4

                def emit_head(sb, fsg, fd4):
                    blocks, runs = sb['blocks'], sb['runs']
                    c0, nco = sb['col0'], sb['ncols']
                    u = up.tile([128, CMAX, HID], dt.bfloat16, tag="u")
                    for (k, b, R, g0) in runs:
                        j = blocks.index(b)
                        lc = g0 - c0
                        nc.vector.tensor_tensor(
                            out=u[:, lc:lc + R, :],
                            in0=fsg[:, lc:lc + R, :],
                            in1=fd4[:, j, :].unsqueeze(1).broadcast_to(
                                [128, R, HID]),
                            op=mybir.AluOpType.add)
                    nc.scalar.activation(out=u[:, 0:nco, :], in_=u[:, 0:nco, :],
                                         func=mybir.ActivationFunctionType.Prelu,
                                         alpha=ALPHA)
                    return u

                def emit_tailA(sb, fsg, u):
                    blocks, runs = sb['blocks'], sb['runs']
                    c0, nco = sb['col0'], sb['ncols']
                    pn = cp.tile([128, CMAX, 8], dt.float32, tag="pn")
                    lg = cp.tile([128, CMAX, 4], dt.float32, tag="lg")
                    exb = cp.tile([128, CMAX, 4], dt.bfloat16, tag="exb")
                    for h in range(HEADS):
                        kp = int(pos_cnt[l, h])
                        kz = int(zero_cnt[l, h])
                        if kp > 0:
                            nc.vector.tensor_reduce(
                                out=pn[:, 0:nco, h],
                                in_=u[:, 0:nco, h * OUT:h * OUT + kp],
                                axis=mybir.AxisListType.X, op=mybir.AluOpType.add)
                        else:
                            nc.vector.memset(pn[:, 0:nco, h], 0.0)
                        if kp + kz < OUT:
                            nc.vector.tensor_reduce(
                                out=pn[:, 0:nco, 4 + h],
                                in_=u[:, 0:nco, h * OUT + kp + kz:(h + 1) * OUT],
                                axis=mybir.AxisListType.X, op=mybir.AluOpType.add)
                        else:
                            nc.vector.memset(pn[:, 0:nco, 4 + h], 0.0)
                    nc.vector.tensor_tensor(out=lg[:, 0:nco, :],
                                            in0=pn[:, 0:nco, 0:4],
                                            in1=pn[:, 0:nco, 4:8],
                                            op=mybir.AluOpType.subtract)
                    nc.vector.tensor_tensor(
                        out=lg[:, 0:nco, :], in0=lg[:, 0:nco, :],
                        in1=maskt[:, c0:c0 + nco].unsqueeze(2).broadcast_to(
                            [128, nco, 4]),
                        op=mybir.AluOpType.add)
                    nc.scalar.activation(out=exb[:, 0:nco, :],
                                         in_=lg[:, 0:nco, :],
                                         func=mybir.ActivationFunctionType.Exp)
                    return exb

                def emit_tailB(sb, fsg, u, exb):
                    blocks, runs = sb['blocks'], sb['runs']
                    c0, nco = sb['col0'], sb['ncols']
                    m = cp.tile([128, CMAX, HID + 4], dt.bfloat16, tag="m")
                    nc.vector.tensor_tensor(
                        out=m[:, 0:nco, 0:HID].rearrange(
                            "p t (h d) -> p t h d", h=HEADS),
                        in0=fsg[:, 0:nco, :].rearrange(
                            "p t (h d) -> p t h d", h=HEADS),
                        in1=exb[:, 0:nco, :].unsqueeze(3).broadcast_to(
                            [128, nco, HEADS, OUT]),
                        op=mybir.AluOpType.mult)
                    nc.vector.tensor_copy(out=m[:, 0:nco, HID:HID + 4],
                                          in_=exb[:, 0:nco, :])

                    if not last:
                        prfs = sp.tile([128, SBMAXB, HID], dt.bfloat16, tag="fs_sb")
                        prfd = sp.tile([128, SBMAXB, HID], dt.bfloat16, tag="fd_sb")
                    nb = len(blocks)
                    rsd = cp.tile([128, SBMAXB, HID + 4], dt.bfloat16, tag="rsd")
                    for j, b in enumerate(blocks):
                        bruns = [(k, R, g0) for (k, bb, R, g0) in runs if bb == b]
                        cols = [g0 - c0 + r for (k, R, g0) in bruns
                                for r in range(R)]
                        if len(cols) == 1:
                            nc.vector.tensor_copy(out=rsd[:, j, :],
                                                  in_=m[:, cols[0], :])
                        else:
                            nc.vector.tensor_tensor(
                                out=rsd[:, j, :], in0=m[:, cols[0], :],
                                in1=m[:, cols[1], :], op=mybir.AluOpType.add)
                            for cc2 in cols[2:]:
                                nc.vector.tensor_tensor(
                                    out=rsd[:, j, :], in0=rsd[:, j, :],
                                    in1=m[:, cc2, :], op=mybir.AluOpType.add)
                    recs = cp.tile([128, SBMAXB, 4], dt.float32, tag="recs")
                    nc.vector.tensor_scalar(
                        out=recs[:, 0:nb, :], in0=rsd[:, 0:nb, HID:HID + 4],
                        scalar1=1e-30, scalar2=None, op0=mybir.AluOpType.max)
                    nc.vector.reciprocal(out=recs[:, 0:nb, :],
                                         in_=recs[:, 0:nb, :])
                    for j, b in enumerate(blocks):
                        hn = bp.tile([128, HID],
                                     dt.float32 if last else dt.bfloat16, tag="hn")
                        nc.vector.tensor_tensor(
                            out=hn[:].rearrange("p (h d) -> p h d", h=HEADS),
                            in0=rsd[:, j, 0:HID].rearrange(
                                "p (h d) -> p h d", h=HEADS),
                            in1=recs[:, j, :].unsqueeze(2).broadcast_to(
                                [128, HEADS, OUT]),
                            op=mybir.AluOpType.mult)
                        if last:
                            nc.sync.dma_start(out_d[b * 128:(b + 1) * 128, :],
                                              hn[:])
                        else:
                            for cch in range(2):
                                pt = psT.tile([128, 128], dt.bfloat16, tag="tp")
                                nc.tensor.transpose(
                                    pt[:], hn[:, cch * 128:(cch + 1) * 128],
                                    ident[:])
                                nc.scalar.activation(
                                    out=hT[:, cch, b * 128:(b + 1) * 128],
                                    in_=pt[:],
                                    func=mybir.ActivationFunctionType.Copy)
                            emit_proj(l + 1, b, nbuf, prfs, prfd, j)
                    if not last:
                        emit_proj_dma(nbuf, blocks, prfs, prfd)

                prev = None
                for sb in sbs:
                    fsg, fd4 = emit_gather(sb)
                    if prev is not None:
                        exb = emit_tailA(*prev)
                    u = emit_head(sb, fsg, fd4)
                    if prev is not None:
                        emit_tailB(*prev, exb)
                    prev = (sb, fsg, u)
                exb = emit_tailA(*prev)
                emit_tailB(*prev, exb)

                # empty blocks: h = 0 for next layer, then project them
                if not last:
                    for i0 in range(0, len(empty_blocks), 8):
                        grp = empty_blocks[i0:i0 + 8]
                        efs = sp.tile([128, 8, HID], dt.bfloat16, tag="fs_sb")
                        efd = sp.tile([128, 8, HID], dt.bfloat16, tag="fd_sb")
                        for j, b in enumerate(grp):
                            nc.vector.memset(hT[:, 0, b * 128:(b + 1) * 128], 0.0)
                            nc.vector.memset(hT[:, 1, b * 128:(b + 1) * 128], 0.0)
                            emit_proj(l + 1, b, nbuf, efs, efd, j)
                        emit_proj_dma(nbuf, grp, efs, efd)
    nc.compile()
    return nc


def kernel(**inputs):
    from concourse.bass_utils import run_bass_kernel_spmd

    src = np.asarray(inputs['src'])
    dst = np.asarray(inputs['dst'])
    atom = np.asarray(inputs['atom_feat']).astype(np.float32)
    Ws_eff, Wd_eff, bs_eff, bd_eff, pos_cnt, zero_cnt, T2inv = _fold_weights(
        inputs['W_in'], inputs['b_in'], inputs['W_src'], inputs['b_src'],
        inputs['W_dst'], inputs['b_dst'], inputs['attn'], inputs['bias'])
    for l in range(LAYERS):
        assert np.abs(bs_eff[l]).max() < 1e-12 and np.abs(bd_eff[l]).max() < 1e-12, \
            "nonzero GAT biases not supported by this kernel build"

    Rtemp, assigns = build_structure(src, dst)
    slots = build_slots(src, dst, Rtemp, assigns)
    NBE, SLAB = slots['NBE'], slots['NBE'] * 128
    posmap = slots['posmap']

    win_np = np.zeros((ATOM_DIM + 1, HID), np.float32)
    win_np[:ATOM_DIM] = np.asarray(inputs['W_in'])
    win_np[ATOM_DIM] = np.asarray(inputs['b_in'])
    wsd_np = np.zeros((128, 2 * LAYERS, 512), np.float32)
    for l in range(LAYERS):
        for kc in range(2):
            wsd_np[:, l * 2 + kc, 0:HID] = Ws_eff[l][kc * 128:(kc + 1) * 128]
            wsd_np[:, l * 2 + kc, HID:512] = Wd_eff[l][kc * 128:(kc + 1) * 128]

    nc = _build(slots, pos_cnt, zero_cnt)

    ident = np.eye(128, dtype=np.float32)
    in_maps = []
    for c in range(NCORES):
        at = np.zeros((ATOM_DIM + 1, SLAB), np.float32)
        atc = atom[c * NPC:(c + 1) * NPC]
        at[:ATOM_DIM, posmap[c]] = atc.T
        at[ATOM_DIM, posmap[c]] = 1.0
        in_maps.append({
            'atomT': _bf(at), 'win': _bf(win_np), 'wsd': _bf(wsd_np),
            'fsi': _wrap16(slots['fsidx'][c]),
            'maskt': _bf(slots['mask'][c]),
            'ident': _bf(ident),
        })
    import os
    res = run_bass_kernel_spmd(nc, in_maps, core_ids=list(range(NCORES)),
                               trace=bool(os.environ.get('KBT_TRACE')))
    kernel._last = res
    out = np.zeros((N, HID), np.float64)
    for c in range(NCORES):
        out[c * NPC:(c + 1) * NPC] = res.results[c]['out'][posmap[c]]
    # zero rows for nodes in empty blocks (their h is exactly 0; device
    # never writes those rows)
    if slots['empty_blocks']:
        emptyset = np.zeros(SLAB, bool)
        for b in slots['empty_blocks']:
            emptyset[b * 128:(b + 1) * 128] = True
        for c in range(NCORES):
            zn = emptyset[posmap[c]]
            out[c * NPC:(c + 1) * NPC][zn] = 0.0
    out = out @ T2inv + np.asarray(inputs['bias'])[LAYERS - 1][None]
    return out.astype(np.float32)


if __name__ == '__main__':
    import jax
    with jax.default_device(jax.devices('cpu')[0]):
        import reference
        inputs = {k: np.asarray(v) for k, v in reference.setup_inputs().items()}
    got = kernel(**inputs)
    print("kernel out:", got.shape, got.dtype, np.abs(got).mean())


# revision 28
# speedup vs baseline: 1.1233x; 1.0335x over previous
"""Trainium2 8-core GATv2 message-passing kernel (nn_AtomGraphEncoder).

Dense (block, round, lane) design:
- Nodes sharded 8 x 12500 by id; WITHIN each core, nodes are permuted into
  blocks of 128 lanes with an SPMD-uniform per-class template R[b, k]
  (class k of an edge = src // 25000 = core-pair window, int16-gatherable).
- Per layer: proj fs/fd on PE (fd -> DRAM fd_tab, fs -> AllGather fs_full),
  then per superblock: one dma_gather per class window, dense free-dim ops:
  u = Prelu(fsg + fd bcast), sign-split reduces for GATv2 logits (|a| folded
  into W on host), exp (+pad mask -1e30), m = [ex*fs | ex], per-(b,k) strided
  reduce over rounds + per-block class-sum -> normalize -> hT / out.
- No scatter-add, no DRAM accumulators, no fd gather.
- Host: weight folding (undone on output), node permutation (undone on host).
"""
import sys

import numpy as np
import ml_dtypes

sys.path.insert(0, '/opt/trn_rl_repo')

N, E = 100000, 400000
ATOM_DIM, HID, LAYERS, HEADS = 74, 256, 3, 4
OUT = HID // HEADS
NCORES = 8
NPC = N // NCORES
W = 25000
NCLS = 4
BF = ml_dtypes.bfloat16
EPS = 1e-20
MASKNEG = -1e30


def _bf(x):
    return np.asarray(x).astype(BF)


def _fold_weights(W_in, b_in, W_src, b_src, W_dst, b_dst, attn, bias):
    Ts, Tinvs = [], []
    pos_cnt = np.zeros((LAYERS, HEADS), np.int64)
    zero_cnt = np.zeros((LAYERS, HEADS), np.int64)
    for l in range(LAYERS):
        Tl = np.zeros((HID, HID), np.float64)
        Tinv = np.zeros((HID, HID), np.float64)
        for h in range(HEADS):
            a = np.asarray(attn)[l, h].astype(np.float64)
            order = np.concatenate([
                np.where(a > 0)[0], np.where(a == 0)[0], np.where(a < 0)[0]])
            pos_cnt[l, h] = (a > 0).sum()
            zero_cnt[l, h] = (a == 0).sum()
            for j, p in enumerate(order):
                s = abs(a[p]) if a[p] != 0 else 1.0
                Tl[h * OUT + p, h * OUT + j] = s
                Tinv[h * OUT + j, h * OUT + p] = 1.0 / s
        Ts.append(Tl)
        Tinvs.append(Tinv)
    Ws_eff, Wd_eff, bs_eff, bd_eff = [], [], [], []
    for l in range(LAYERS):
        Tp = np.eye(HID) if l == 0 else Tinvs[l - 1]
        Ws = np.asarray(W_src)[l].astype(np.float64)
        Wd = np.asarray(W_dst)[l].astype(np.float64)
        bprev = np.zeros(HID) if l == 0 else np.asarray(bias)[l - 1].astype(np.float64)
        Ws_eff.append((Tp @ Ws @ Ts[l]).astype(np.float32))
        Wd_eff.append((Tp @ Wd @ Ts[l]).astype(np.float32))
        bs_eff.append(((np.asarray(b_src)[l] + bprev @ Ws) @ Ts[l]).astype(np.float32))
        bd_eff.append(((np.asarray(b_dst)[l] + bprev @ Wd) @ Ts[l]).astype(np.float32))
    return Ws_eff, Wd_eff, bs_eff, bd_eff, pos_cnt, zero_cnt, Tinvs[-1]


# ---------------------------------------------------------------- host prep
CMAX = 32           # max columns per superblock tile
SBMAXB = 8          # max blocks per superblock


def _percore_edges(src, dst):
    out = []
    for c in range(NCORES):
        m = (dst >= c * NPC) & (dst < (c + 1) * NPC)
        out.append((src[m], dst[m] - c * NPC))
    return out


def _class_counts(es, ed):
    cnt = np.zeros((NPC, NCLS), np.int64)
    np.add.at(cnt, (ed, es // W), 1)
    return cnt


def _lex_order(cnt):
    return np.lexsort((-cnt[:, 0], -cnt[:, 1], -cnt[:, 2], -cnt[:, 3]))


def _block_R(cnt, order, nb):
    cs = cnt[order]
    cs = np.vstack([cs, np.zeros((nb * 128 - len(cs), NCLS), np.int64)])
    return cs.reshape(nb, 128, NCLS).max(1)


def build_structure(src, dst, quant=0.40):
    src = np.asarray(src).astype(np.int64)
    dst = np.asarray(dst).astype(np.int64)
    percore = _percore_edges(src, dst)
    cnts = [_class_counts(es, ed) for es, ed in percore]
    NB = (NPC + 127) // 128

    Rs = np.stack([_block_R(cnts[c], _lex_order(cnts[c]), NB) for c in range(NCORES)])
    Rtemp = np.ceil(np.quantile(Rs, quant, axis=0)).astype(np.int64)

    assigns = np.full((NCORES, NB, 128), -1, np.int64)
    overflows = []
    for c in range(NCORES):
        cnt = cnts[c]
        order = _lex_order(cnt)
        cap = np.full(NB, 128, np.int64)
        fill = np.zeros(NB, np.int64)
        ov = []
        for idx in order:
            v = cnt[idx]
            placed = False
            for b in range(NB):
                if cap[b] > 0 and (Rtemp[b] >= v).all():
                    assigns[c, b, fill[b]] = idx
                    fill[b] += 1
                    cap[b] -= 1
                    placed = True
                    break
            if not placed:
                ov.append(idx)
        overflows.append(ov)

    nov_nodes = max(len(o) for o in overflows)
    n_ovb = (nov_nodes + 127) // 128
    if n_ovb:
        Rov = np.zeros((n_ovb, NCLS), np.int64)
        ovassign = np.full((NCORES, n_ovb, 128), -1, np.int64)
        for c in range(NCORES):
            ov = overflows[c]
            if ov:
                ovc = cnts[c][ov]
                o2 = np.lexsort((-ovc[:, 0], -ovc[:, 1], -ovc[:, 2], -ovc[:, 3]))
                ov = [ov[i] for i in o2]
            for i, idx in enumerate(ov):
                b = i // 128
                ovassign[c, b, i % 128] = idx
                np.maximum(Rov[b], cnts[c][idx], out=Rov[b])
        Rtemp = np.vstack([Rtemp, Rov])
        assigns = np.concatenate([assigns, ovassign], axis=1)
    return Rtemp, assigns


def build_slots(src, dst, Rtemp, assigns):
    src = np.asarray(src).astype(np.int64)
    dst = np.asarray(dst).astype(np.int64)
    NBE = Rtemp.shape[0]
    SLAB = NBE * 128

    posmap = np.full((NCORES, NPC), -1, np.int64)
    for c in range(NCORES):
        b_idx, lane_idx = np.nonzero(assigns[c] >= 0)
        nodes = assigns[c][b_idx, lane_idx]
        posmap[c, nodes] = b_idx * 128 + lane_idx

    core_of = np.arange(N) // NPC
    fsrow = core_of * SLAB + posmap[core_of, np.arange(N) % NPC]

    percore = _percore_edges(src, dst)

    Rsum = Rtemp.sum(1)
    sbs = []
    cur, cur_cols = [], 0
    for b in range(NBE):
        if Rsum[b] == 0:
            continue
        if cur and (cur_cols + Rsum[b] > CMAX or len(cur) == SBMAXB):
            sbs.append(cur)
            cur, cur_cols = [], 0
        cur.append(b)
        cur_cols += Rsum[b]
    if cur:
        sbs.append(cur)
    empty_blocks = [b for b in range(NBE) if Rsum[b] == 0]

    sb_meta = []
    gcol = 0
    for blocks in sbs:
        runs = []
        c0 = gcol
        for k in range(NCLS):
            for b in blocks:
                R = int(Rtemp[b, k])
                if R:
                    runs.append((k, b, R, gcol))
                    gcol += R
        sb_meta.append(dict(blocks=blocks, runs=runs, col0=c0, ncols=gcol - c0))
    C = gcol
    S = C * 128

    fsidx = np.zeros((NCORES, S), np.int64)
    mask = np.zeros((NCORES, 128, C), np.float32)
    for c in range(NCORES):
        es, ed = percore[c]
        cls = es // W
        order = np.lexsort((cls, ed))
        es_s, cls_s = es[order], cls[order]
        ed_s = ed[order]
        key = ed_s * NCLS + cls_s
        starts = np.searchsorted(key, np.arange(NPC * NCLS))
        ends = np.searchsorted(key, np.arange(NPC * NCLS), side='right')
        for sb in sb_meta:
            for (k, b, R, g0) in sb['runs']:
                for lane in range(128):
                    node = assigns[c, b, lane]
                    if node >= 0:
                        s0, s1 = starts[node * NCLS + k], ends[node * NCLS + k]
                        nn = min(R, s1 - s0)
                        for r in range(nn):
                            fsidx[c, (g0 + r) * 128 + lane] = \
                                fsrow[es_s[s0 + r]] - k * 2 * SLAB
                        for r in range(nn, R):
                            mask[c, lane, g0 + r] = MASKNEG
                    else:
                        for r in range(R):
                            mask[c, lane, g0 + r] = MASKNEG
    return dict(NBE=NBE, sbs=sb_meta, empty_blocks=empty_blocks, C=C, S=S,
                fsidx=fsidx, mask=mask, posmap=posmap)


def _wrap16(idx):
    w = np.ascontiguousarray(np.asarray(idx).reshape(-1, 16).T).astype(np.int16)
    return np.tile(w, (8, 1))


# ---------------------------------------------------------------- bass build
def _build(slots, pos_cnt, zero_cnt):
    import concourse.bass as bass
    import concourse.tile as tile
    from concourse import bacc, mybir, library_config

    NBE, sbs, C, S = slots['NBE'], slots['sbs'], slots['C'], slots['S']
    empty_blocks = slots['empty_blocks']
    SLAB = NBE * 128
    ALPHA = 0.2

    nc = bacc.Bacc("TRN2", target_bir_lowering=False, debug=False,
                   num_devices=NCORES)
    dt = mybir.dt
    atomT_d = nc.dram_tensor("atomT", [ATOM_DIM + 1, SLAB], dt.bfloat16,
                             kind="ExternalInput")
    win_d = nc.dram_tensor("win", [ATOM_DIM + 1, HID], dt.bfloat16,
                           kind="ExternalInput")
    wsd_d = nc.dram_tensor("wsd", [128, 2 * LAYERS, 512], dt.bfloat16,
                           kind="ExternalInput")
    fsi_d = nc.dram_tensor("fsi", [128, S // 16], dt.int16, kind="ExternalInput")
    mask_d = nc.dram_tensor("maskt", [128, C], dt.bfloat16, kind="ExternalInput")
    ident_d = nc.dram_tensor("ident", [128, 128], dt.bfloat16, kind="ExternalInput")
    out_d = nc.dram_tensor("out", [SLAB, HID], dt.float32, kind="ExternalOutput")

    fs_bounce = [nc.dram_tensor(f"fs_bounce{i}", [SLAB, HID], dt.bfloat16)
                 for i in range(2)]
    fd_tab = [nc.dram_tensor(f"fd_tab{i}", [SLAB, HID], dt.bfloat16)
              for i in range(2)]
    fs_full = [nc.dram_tensor(f"fs_full{i}", [NCORES * SLAB, HID], dt.bfloat16,
                              addr_space="Shared") for i in range(2)]

    with tile.TileContext(nc) as tc:
        nc.gpsimd.load_library(library_config.mlp)
        with tc.tile_pool(name="persist", bufs=1) as pp, \
             tc.tile_pool(name="gat", bufs=4) as gp, \
             tc.tile_pool(name="fdp", bufs=2) as fp, \
             tc.tile_pool(name="upool", bufs=2) as up, \
             tc.tile_pool(name="cmp", bufs=1) as cp, \
             tc.tile_pool(name="blk", bufs=2) as bp, \
             tc.tile_pool(name="stage", bufs=2) as sp, \
             tc.tile_pool(name="psA", bufs=2, space="PSUM") as psA, \
             tc.tile_pool(name="psT", bufs=2, space="PSUM") as psT:

            fsi = pp.tile([128, S // 16], dt.int16, tag="fsi")
            maskt = pp.tile([128, C], dt.bfloat16, tag="maskt")
            wsd = pp.tile([128, 2 * LAYERS, 512], dt.bfloat16, tag="wsd")
            win = pp.tile([ATOM_DIM + 1, HID], dt.bfloat16, tag="win")
            ident = pp.tile([128, 128], dt.bfloat16, tag="ident")
            hT = pp.tile([128, 2, SLAB], dt.bfloat16, tag="hT")
            nc.sync.dma_start(fsi[:], fsi_d[:])
            nc.sync.dma_start(maskt[:], mask_d[:])
            nc.sync.dma_start(wsd[:], wsd_d[:])
            nc.sync.dma_start(win[:], win_d[:])
            nc.sync.dma_start(ident[:], ident_d[:])

            def emit_proj(lw, a, pbuf, stfs, stfd, j, on_dve=False):
                """Project block a with layer-lw weights into staging col j."""
                ps = psA.tile([128, 512], dt.float32, tag="projps")
                for kc in range(2):
                    nc.tensor.matmul(
                        ps[:],
                        hT[:, kc, a * 128:(a + 1) * 128],
                        wsd[:, lw * 2 + kc, :],
                        start=(kc == 0), stop=(kc == 1))
                if on_dve:
                    nc.vector.tensor_copy(out=stfs[:, j, :], in_=ps[:, 0:HID])
                    nc.vector.tensor_copy(out=stfd[:, j, :], in_=ps[:, HID:512])
                else:
                    nc.scalar.activation(out=stfs[:, j, :], in_=ps[:, 0:HID],
                                         func=mybir.ActivationFunctionType.Copy)
                    nc.scalar.activation(out=stfd[:, j, :], in_=ps[:, HID:512],
                                         func=mybir.ActivationFunctionType.Copy)

            def emit_proj_dma(pbuf, blist, stfs, stfd):
                """DMA staged proj rows for blocks blist (consecutive or not)."""
                nb = len(blist)
                if blist == list(range(blist[0], blist[0] + nb)):
                    b0 = blist[0]
                    nc.sync.dma_start(
                        fs_bounce[pbuf][b0 * 128:(b0 + nb) * 128, :].rearrange(
                            "(a p) e -> p a e", p=128), stfs[:, 0:nb, :])
                    nc.sync.dma_start(
                        fd_tab[pbuf][b0 * 128:(b0 + nb) * 128, :].rearrange(
                            "(a p) e -> p a e", p=128), stfd[:, 0:nb, :])
                else:
                    for j, b in enumerate(blist):
                        nc.sync.dma_start(fs_bounce[pbuf][b * 128:(b + 1) * 128, :],
                                          stfs[:, j, :])
                        nc.sync.dma_start(fd_tab[pbuf][b * 128:(b + 1) * 128, :],
                                          stfd[:, j, :])

            # ---- input projection fused with layer-0 projection per chunk
            CH = 1024
            s0 = 0
            while s0 < SLAB:
                w_all = min(CH, SLAB - s0)
                atc = sp.tile([ATOM_DIM + 1, CH], dt.bfloat16, tag="atc")
                nc.sync.dma_start(atc[:, 0:w_all], atomT_d[:, s0:s0 + w_all])
                t = 0
                while t < w_all:
                    w_ = min(512, w_all - t)
                    for cch in range(2):
                        ps = psA.tile([128, 512], dt.float32, tag="projps")
                        nc.tensor.matmul(ps[:, 0:w_],
                                         win[:, cch * 128:(cch + 1) * 128],
                                         atc[:, t:t + w_],
                                         start=True, stop=True)
                        nc.vector.tensor_copy(out=hT[:, cch, s0 + t:s0 + t + w_],
                                               in_=ps[:, 0:w_])
                    t += w_
                nblk = w_all // 128
                b0 = s0 // 128
                fs_sb = sp.tile([128, 8, HID], dt.bfloat16, tag="fs_sb")
                fd_sb = sp.tile([128, 8, HID], dt.bfloat16, tag="fd_sb")
                for j in range(nblk):
                    emit_proj(0, b0 + j, 0, fs_sb, fd_sb, j, on_dve=True)
                emit_proj_dma(0, list(range(b0, b0 + nblk)), fs_sb, fd_sb)
                s0 += w_all

            for l in range(LAYERS):
                last = l == LAYERS - 1
                pbuf = l % 2
                nbuf = (l + 1) % 2

                # ---- AllGather fs table for this layer
                nc.gpsimd.collective_compute(
                    "AllGather", mybir.AluOpType.bypass,
                    replica_groups=[list(range(NCORES))],
                    ins=[fs_bounce[pbuf][:].opt()], outs=[fs_full[pbuf][:].opt()])

                # ---- per-superblock pipeline: [gather i][tail i-1][head i]
                def emit_gather(sb):
                    blocks, runs = sb['blocks'], sb['runs']
                    c0 = sb['col0']
                    fsg = gp.tile([128, CMAX, HID], dt.bfloat16, tag="fsg")
                    fd4 = fp.tile([128, SBMAXB, HID], dt.bfloat16, tag="fd4")
                    kruns = {}
                    for (k, b, R, g0) in runs:
                        lo, n = kruns.get(k, (g0, 0))
                        kruns[k] = (min(lo, g0), n + R)
                    for k, (g0, ncols_k) in sorted(kruns.items()):
                        lo = k * 2 * SLAB
                        hi = min(lo + 2 * SLAB, NCORES * SLAB)
                        for q0 in range(0, ncols_k, 16):
                            qn = min(16, ncols_k - q0)
                            g = g0 + q0
                            nc.gpsimd.dma_gather(
                                fsg[:, g - c0:g - c0 + qn, :],
                                fs_full[pbuf][lo:hi, :],
                                fsi[:, g * 8:(g + qn) * 8], qn * 128, qn * 128,
                                HID, single_packet=False)
                    nb = len(blocks)
                    if blocks == list(range(blocks[0], blocks[0] + nb)):
                        b0 = blocks[0]
                        nc.sync.dma_start(
                            fd4[:, 0:nb, :],
                            fd_tab[pbuf][b0 * 128:(b0 + nb) * 128, :].rearrange(
                                "(a p) e -> p a e", p=128))
                    else:
                        for j, b in enumerate(blocks):
                            nc.sync.dma_start(fd4[:, j, :],
                                              fd_tab[pbuf][b * 128:(b + 1) * 128, :])
                    return fsg, fd4

                def emit_head(sb, fsg, fd4):
                    blocks, runs = sb['blocks'], sb['runs']
                    c0, nco = sb['col0'], sb['ncols']
                    u = up.tile([128, CMAX, HID], dt.bfloat16, tag="u")
                    for (k, b, R, g0) in runs:
                        j = blocks.index(b)
                        lc = g0 - c0
                        nc.vector.tensor_tensor(
                            out=u[:, lc:lc + R, :],
                            in0=fsg[:, lc:lc + R, :],
                            in1=fd4[:, j, :].unsqueeze(1).broadcast_to(
                                [128, R, HID]),
                            op=mybir.AluOpType.add)
                    nc.scalar.activation(out=u[:, 0:nco, :], in_=u[:, 0:nco, :],
                                         func=mybir.ActivationFunctionType.Prelu,
                                         alpha=ALPHA)
                    return u

                def emit_tailA(sb, fsg, u):
                    blocks, runs = sb['blocks'], sb['runs']
                    c0, nco = sb['col0'], sb['ncols']
                    pn = cp.tile([128, CMAX, 8], dt.float32, tag="pn")
                    lg = cp.tile([128, CMAX, 4], dt.float32, tag="lg")
                    exb = cp.tile([128, CMAX, 4], dt.bfloat16, tag="exb")
                    for h in range(HEADS):
                        kp = int(pos_cnt[l, h])
                        kz = int(zero_cnt[l, h])
                        if kp > 0:
                            nc.vector.tensor_reduce(
                                out=pn[:, 0:nco, h],
                                in_=u[:, 0:nco, h * OUT:h * OUT + kp],
                                axis=mybir.AxisListType.X, op=mybir.AluOpType.add)
                        else:
                            nc.vector.memset(pn[:, 0:nco, h], 0.0)
                        if kp + kz < OUT:
                            nc.vector.tensor_reduce(
                                out=pn[:, 0:nco, 4 + h],
                                in_=u[:, 0:nco, h * OUT + kp + kz:(h + 1) * OUT],
                                axis=mybir.AxisListType.X, op=mybir.AluOpType.add)
                        else:
                            nc.vector.memset(pn[:, 0:nco, 4 + h], 0.0)
                    nc.vector.tensor_tensor(out=lg[:, 0:nco, :],
                                            in0=pn[:, 0:nco, 0:4],
                                            in1=pn[:, 0:nco, 4:8],
                                            op=mybir.AluOpType.subtract)
                    nc.vector.tensor_tensor(
                        out=lg[:, 0:nco, :], in0=lg[:, 0:nco, :],
                        in1=maskt[:, c0:c0 + nco].unsqueeze(2).broadcast_to(
                            [128, nco, 4]),
                        op=mybir.AluOpType.add)
                    nc.scalar.activation(out=exb[:, 0:nco, :],
                                         in_=lg[:, 0:nco, :],
                                         func=mybir.ActivationFunctionType.Exp)
                    return exb

                def emit_tailB(sb, fsg, u, exb):
                    blocks, runs = sb['blocks'], sb['runs']
                    c0, nco = sb['col0'], sb['ncols']
                    m = up.tile([128, CMAX, HID + 4], dt.bfloat16, tag="u")
                    nc.vector.tensor_tensor(
                        out=m[:, 0:nco, 0:HID].rearrange(
                            "p t (h d) -> p t h d", h=HEADS),
                        in0=fsg[:, 0:nco, :].rearrange(
                            "p t (h d) -> p t h d", h=HEADS),
                        in1=exb[:, 0:nco, :].unsqueeze(3).broadcast_to(
                            [128, nco, HEADS, OUT]),
                        op=mybir.AluOpType.mult)
                    nc.vector.tensor_copy(out=m[:, 0:nco, HID:HID + 4],
                                          in_=exb[:, 0:nco, :])

                    if not last:
                        prfs = sp.tile([128, SBMAXB, HID], dt.bfloat16, tag="fs_sb")
                        prfd = sp.tile([128, SBMAXB, HID], dt.bfloat16, tag="fd_sb")
                    nb = len(blocks)
                    rsd = cp.tile([128, SBMAXB, HID + 4], dt.bfloat16, tag="rsd")
                    for j, b in enumerate(blocks):
                        bruns = [(k, R, g0) for (k, bb, R, g0) in runs if bb == b]
                        cols = [g0 - c0 + r for (k, R, g0) in bruns
                                for r in range(R)]
                        if len(cols) == 1:
                            nc.vector.tensor_copy(out=rsd[:, j, :],
                                                  in_=m[:, cols[0], :])
                        else:
                            nc.vector.tensor_tensor(
                                out=rsd[:, j, :], in0=m[:, cols[0], :],
                                in1=m[:, cols[1], :], op=mybir.AluOpType.add)
                            for cc2 in cols[2:]:
                                nc.vector.tensor_tensor(
                                    out=rsd[:, j, :], in0=rsd[:, j, :],
                                    in1=m[:, cc2, :], op=mybir.AluOpType.add)
                    recs = cp.tile([128, SBMAXB, 4], dt.float32, tag="recs")
                    nc.vector.tensor_scalar(
                        out=recs[:, 0:nb, :], in0=rsd[:, 0:nb, HID:HID + 4],
                        scalar1=1e-30, scalar2=None, op0=mybir.AluOpType.max)
                    nc.vector.reciprocal(out=recs[:, 0:nb, :],
                                         in_=recs[:, 0:nb, :])
                    for j, b in enumerate(blocks):
                        hn = bp.tile([128, HID],
                                     dt.float32 if last else dt.bfloat16, tag="hn")
                        nc.vector.tensor_tensor(
                            out=hn[:].rearrange("p (h d) -> p h d", h=HEADS),
                            in0=rsd[:, j, 0:HID].rearrange(
                                "p (h d) -> p h d", h=HEADS),
                            in1=recs[:, j, :].unsqueeze(2).broadcast_to(
                                [128, HEADS, OUT]),
                            op=mybir.AluOpType.mult)
                        if last:
                            nc.sync.dma_start(out_d[b * 128:(b + 1) * 128, :],
                                              hn[:])
                        else:
                            for cch in range(2):
                                pt = psT.tile([128, 128], dt.bfloat16, tag="tp")
                                nc.tensor.transpose(
                                    pt[:], hn[:, cch * 128:(cch + 1) * 128],
                                    ident[:])
                                nc.scalar.activation(
                                    out=hT[:, cch, b * 128:(b + 1) * 128],
                                    in_=pt[:],
                                    func=mybir.ActivationFunctionType.Copy)
                            emit_proj(l + 1, b, nbuf, prfs, prfd, j)
                    if not last:
                        emit_proj_dma(nbuf, blocks, prfs, prfd)

                prev = None
                for sb in sbs:
                    fsg, fd4 = emit_gather(sb)
                    if prev is not None:
                        exb = emit_tailA(*prev)
                    u = emit_head(sb, fsg, fd4)
                    if prev is not None:
                        emit_tailB(*prev, exb)
                    prev = (sb, fsg, u)
                exb = emit_tailA(*prev)
                emit_tailB(*prev, exb)

                # empty blocks: h = 0 for next layer, then project them
                if not last:
                    for i0 in range(0, len(empty_blocks), 8):
                        grp = empty_blocks[i0:i0 + 8]
                        efs = sp.tile([128, 8, HID], dt.bfloat16, tag="fs_sb")
                        efd = sp.tile([128, 8, HID], dt.bfloat16, tag="fd_sb")
                        for j, b in enumerate(grp):
                            nc.vector.memset(hT[:, 0, b * 128:(b + 1) * 128], 0.0)
                            nc.vector.memset(hT[:, 1, b * 128:(b + 1) * 128], 0.0)
                            emit_proj(l + 1, b, nbuf, efs, efd, j)
                        emit_proj_dma(nbuf, grp, efs, efd)
    nc.compile()
    return nc


def kernel(**inputs):
    from concourse.bass_utils import run_bass_kernel_spmd

    src = np.asarray(inputs['src'])
    dst = np.asarray(inputs['dst'])
    atom = np.asarray(inputs['atom_feat']).astype(np.float32)
    Ws_eff, Wd_eff, bs_eff, bd_eff, pos_cnt, zero_cnt, T2inv = _fold_weights(
        inputs['W_in'], inputs['b_in'], inputs['W_src'], inputs['b_src'],
        inputs['W_dst'], inputs['b_dst'], inputs['attn'], inputs['bias'])
    for l in range(LAYERS):
        assert np.abs(bs_eff[l]).max() < 1e-12 and np.abs(bd_eff[l]).max() < 1e-12, \
            "nonzero GAT biases not supported by this kernel build"

    Rtemp, assigns = build_structure(src, dst)
    slots = build_slots(src, dst, Rtemp, assigns)
    NBE, SLAB = slots['NBE'], slots['NBE'] * 128
    posmap = slots['posmap']

    win_np = np.zeros((ATOM_DIM + 1, HID), np.float32)
    win_np[:ATOM_DIM] = np.asarray(inputs['W_in'])
    win_np[ATOM_DIM] = np.asarray(inputs['b_in'])
    wsd_np = np.zeros((128, 2 * LAYERS, 512), np.float32)
    for l in range(LAYERS):
        for kc in range(2):
            wsd_np[:, l * 2 + kc, 0:HID] = Ws_eff[l][kc * 128:(kc + 1) * 128]
            wsd_np[:, l * 2 + kc, HID:512] = Wd_eff[l][kc * 128:(kc + 1) * 128]

    nc = _build(slots, pos_cnt, zero_cnt)

    ident = np.eye(128, dtype=np.float32)
    in_maps = []
    for c in range(NCORES):
        at = np.zeros((ATOM_DIM + 1, SLAB), np.float32)
        atc = atom[c * NPC:(c + 1) * NPC]
        at[:ATOM_DIM, posmap[c]] = atc.T
        at[ATOM_DIM, posmap[c]] = 1.0
        in_maps.append({
            'atomT': _bf(at), 'win': _bf(win_np), 'wsd': _bf(wsd_np),
            'fsi': _wrap16(slots['fsidx'][c]),
            'maskt': _bf(slots['mask'][c]),
            'ident': _bf(ident),
        })
    import os
    res = run_bass_kernel_spmd(nc, in_maps, core_ids=list(range(NCORES)),
                               trace=bool(os.environ.get('KBT_TRACE')))
    kernel._last = res
    out = np.zeros((N, HID), np.float64)
    for c in range(NCORES):
        out[c * NPC:(c + 1) * NPC] = res.results[c]['out'][posmap[c]]
    # zero rows for nodes in empty blocks (their h is exactly 0; device
    # never writes those rows)
    if slots['empty_blocks']:
        emptyset = np.zeros(SLAB, bool)
        for b in slots['empty_blocks']:
            emptyset[b * 128:(b + 1) * 128] = True
        for c in range(NCORES):
            zn = emptyset[posmap[c]]
            out[c * NPC:(c + 1) * NPC][zn] = 0.0
    out = out @ T2inv + np.asarray(inputs['bias'])[LAYERS - 1][None]
    return out.astype(np.float32)


if __name__ == '__main__':
    import jax
    with jax.default_device(jax.devices('cpu')[0]):
        import reference
        inputs = {k: np.asarray(v) for k, v in reference.setup_inputs().items()}
    got = kernel(**inputs)
    print("kernel out:", got.shape, got.dtype, np.abs(got).mean())
